# revision 39
# baseline (speedup 1.0000x reference)
"""Causal multi-head attention on 8 Trainium2 NeuronCores.

Sharding: data-parallel over batch (B=2) x tensor-parallel over heads
(16 heads -> 4 groups of 4). Core c handles batch c//4, heads
[4*(c%4), 4*(c%4)+4). Each core computes its head-slice QKV projections,
causal softmax attention, and a partial output projection (row-sharded
Wo). The host sums the 4 partials per batch and adds the biases that
commute with the reduction (bo + Wo @ bv).

Schedule (v3, ~157us vs 173us baseline):
  - x is DMA'd q-chunk-major (one 8KB-run DMA per chunk, split over 4
    queues) and Q/K are projected one 512-column chunk at a time, so
    attention for chunk 0 starts ~8us in instead of ~30us.
  - the attention inner loop is ACT(exp)-bound; the remaining Q/K
    chunks, the next chunk's v projection and the previous chunk's
    output projection are threaded INTO the attention stream as
    128-mode "filler" matmuls after each p@v burst, keeping the PE
    busy so the HAM clock gate never re-throttles (idle-heavy
    schedules measured 13-24us stretches at 1.2GHz).
  - p@v runs in fp8e4 DoubleRow: one matmul contracts a k-tile PAIR
    (2 weights per PE cell), halving p@v streaming time. exp writes
    pt straight to fp8. The first k-tile pair of chunk 0 stays bf16:
    rows with tiny softmax support (q<256) cannot average away fp8
    noise (measured 3e-2 rel err at row 0 with all-fp8, 1/sqrt(n)
    decay).
  - Q/K/V projections for chunks 1-2 also run fp8 DoubleRow (d-tile
    pairs, K=256/pass). Chunks 0 and 3 stay bf16: chunk 0 for the
    same small-support precision reason, chunk 3 deliberately, as
    cheap PE filler in the ACT-bound tail (thin filler there lets
    the HAM clock gate re-throttle the PE to 1.2GHz).
  - the last chunk's normalize copies and output evacuations run on
    the Scalar engine (idle at the tail) instead of the congested
    DVE queue.
  - diagonal-pair exps are merged into one 3D-AP activation; the
    second tile's stale+triangle region is zeroed with a 256-wide
    [zeros|triangle] mask.
  - the last f-tile's normalize multiply is split per 128 columns so
    the tail output projection starts per-block.
  - partial output is shipped bf16 (halves the out DMA).

Hardware notes baked into this design (all measured on-device):
  - 64x128 row-tiled T0/T8 matmul pairs DO run concurrently
    (108ns/MM) but only in homogeneous instruction streams; tiling-
    mode switches cost ~105ns PE drain each, so 64-mode scores and
    128-mode everything-else are batched.
  - gpsimd partition_broadcast silently fails for output base
    partition 64 (only base-0 tiles work).
  - DVE tensor_tensor cannot read two PSUM operands (one PSUM read
    port) and cannot shift partitions (tensor_copy can).
"""

import os

os.environ.setdefault("MYCRO_LOCAL_CACHE", "1")

import ml_dtypes
import numpy as np

import concourse.bass as bass
import concourse.tile as tile
from concourse import bacc, mybir
from concourse.bass import ds, ts
from concourse.bass_utils import run_bass_kernel_spmd

AF = mybir.ActivationFunctionType

B = 2
S = 2048
D = 1024
N_HEADS = 16
DH = 64
N_CORES = 8

HG = 4            # heads per core
FH = HG * DH      # 256 features per core
P = 128
NFT = FH // P     # 2 f-tiles per core
NDT = D // P      # 8 d_model tiles
QC = 512          # q chunk (moving free dim)
NQC = S // QC     # 4
KT = 128          # k tile (partition dim of sT)
NKT = S // KT     # 16
NEH = D // QC     # 2 output-projection column halves

F32 = mybir.dt.float32
BF16 = mybir.dt.bfloat16
FP8 = mybir.dt.float8e4

MMDT = BF16
PVDT = FP8        # p@v operand dtype (DoubleRow: 2 k-tiles per matmul)
OUTDT = BF16      # partial-output dtype shipped to host
VPAD = 68         # v row padded so the DoubleRow weight step is 16B-aligned


def to_mmdt(a):
    a = np.ascontiguousarray(np.asarray(a, np.float32))
    return np.ascontiguousarray(a.astype(ml_dtypes.bfloat16))


def to_fp8(a):
    a = np.ascontiguousarray(np.asarray(a, np.float32))
    return np.ascontiguousarray(a.astype(ml_dtypes.float8_e4m3fn))


def build_program():
    nc = bacc.Bacc(None, target_bir_lowering=False)

    x_d = nc.dram_tensor("x_img", [P, NDT * S], MMDT, kind="ExternalInput")
    wq_d = nc.dram_tensor("wq_img", [P, NDT * FH], MMDT, kind="ExternalInput")
    wk_d = nc.dram_tensor("wk_img", [P, NDT * FH], MMDT, kind="ExternalInput")
    wv_d = nc.dram_tensor("wv_img", [P, NDT * FH], MMDT, kind="ExternalInput")
    wo_d = nc.dram_tensor("wo_img", [P, NFT * D], MMDT, kind="ExternalInput")
    x8_d = nc.dram_tensor("x8_img", [P, NDT * S], FP8, kind="ExternalInput")
    wq8_d = nc.dram_tensor("wq8_img", [P, NDT * FH], FP8, kind="ExternalInput")
    wk8_d = nc.dram_tensor("wk8_img", [P, NDT * FH], FP8, kind="ExternalInput")
    wv8_d = nc.dram_tensor("wv8_img", [P, NDT * FH], FP8, kind="ExternalInput")
    bq_d = nc.dram_tensor("bq2", [P, NFT], F32, kind="ExternalInput")
    bk_d = nc.dram_tensor("bk2", [P, NFT], F32, kind="ExternalInput")
    out_d = nc.dram_tensor("out", [S, D], OUTDT, kind="ExternalOutput")

    with tile.TileContext(nc) as tc:
        with tc.tile_pool(name="persist", bufs=1) as persist:
            qT = persist.tile([P, NFT, S], MMDT)
            kT = persist.tile([P, NFT, S], MMDT)
            v_sb = persist.tile([P, NKT, HG, VPAD], PVDT)
            v_bf = persist.tile([P, 2, HG, DH + 1], MMDT)
            aTn = persist.tile([P, NFT, S], MMDT)
            wo_sb = persist.tile([P, NFT, D], MMDT)
            bq_sb = persist.tile([P, NFT], F32)
            bk_sb = persist.tile([P, NFT], F32)

            nc.scalar.dma_start(bq_sb[:], bq_d[:])
            nc.scalar.dma_start(bk_sb[:], bk_d[:])
            nc.vector.memset(v_sb[:, :, :, DH : DH + 1], 1.0)
            nc.vector.memset(v_bf[:, :, :, DH : DH + 1], 1.0)

            # triangle mask (keep k<=q) for the causal diagonal, and a
            # [zeros | triangle] double-width variant for the merged
            # diagonal-pair exp (zeroes the stale region + the triangle)
            tri = persist.tile([P, KT], PVDT)
            tri_bf = persist.tile([P, KT], MMDT)
            nc.vector.memset(tri[:], 1.0)
            nc.vector.memset(tri_bf[:], 1.0)
            nc.gpsimd.affine_select(
                out=tri_bf[:],
                in_=tri_bf[:],
                compare_op=mybir.AluOpType.is_ge,
                fill=0.0,
                base=0,
                channel_multiplier=-1,
                pattern=[[1, KT]],
            )
            nc.gpsimd.affine_select(
                out=tri[:],
                in_=tri[:],
                compare_op=mybir.AluOpType.is_ge,
                fill=0.0,
                base=0,
                channel_multiplier=-1,
                pattern=[[1, KT]],
            )
            warm_w = persist.tile([P, KT], MMDT)
            warm_x = persist.tile([P, QC], MMDT)
            nc.vector.memset(warm_w[:], 0.25)
            nc.vector.memset(warm_x[:], 0.25)
            tri2 = persist.tile([P, 2 * KT], PVDT)
            nc.vector.memset(tri2[:], 1.0)
            nc.gpsimd.affine_select(
                out=tri2[:],
                in_=tri2[:],
                compare_op=mybir.AluOpType.is_ge,
                fill=0.0,
                base=-KT,
                channel_multiplier=-1,
                pattern=[[1, 2 * KT]],
            )
            with tc.tile_pool(name="proj", bufs=1) as proj_pool:
                # per-dt weight tiles and chunk-major x tiles: the first
                # projection matmul waits on 64KB of weights + 128KB of x
                wq_dt = [proj_pool.tile([P, FH], MMDT, name=f"wq{dt}") for dt in range(NDT)]
                wk_dt = [proj_pool.tile([P, FH], MMDT, name=f"wk{dt}") for dt in range(NDT)]
                wv_sb = proj_pool.tile([P, NDT, FH], MMDT)
                nc.scalar.dma_start(wq_dt[0][:], wq_d[:, ts(0, FH)])
                nc.scalar.dma_start(wk_dt[0][:], wk_d[:, ts(0, FH)])
                x0 = proj_pool.tile([P, NDT, QC], MMDT, name="x0")
                for part in range(4):
                    nc.sync.dma_start(
                        x0[:, 2 * part : 2 * part + 2, :],
                        x_d[:, ds(2 * part * QC, 2 * QC)].rearrange(
                            "p (dt q) -> p dt q", q=QC
                        ),
                    )
                x3 = proj_pool.tile([P, NDT, QC], MMDT, name="x3")
                for part in range(4):
                    nc.sync.dma_start(
                        x3[:, 2 * part : 2 * part + 2, :],
                        x_d[:, ds((3 * NDT + 2 * part) * QC, 2 * QC)].rearrange(
                            "p (dt q) -> p dt q", q=QC
                        ),
                    )
                xbf = {0: x0, 3: x3}
                x_ct = {c: [xbf[c][:, dt, :] for dt in range(NDT)] for c in (0, 3)}
                x8_ct = [None]
                for c in (1, 2):
                    xc8 = proj_pool.tile([P, NDT, QC], FP8, name=f"x8_{c}")
                    for part in range(2):
                        nc.sync.dma_start(
                            xc8[:, 4 * part : 4 * part + 4, :],
                            x8_d[:, ds((c * NDT + 4 * part) * QC, 4 * QC)].rearrange(
                                "p (dt q) -> p dt q", q=QC
                            ),
                        )
                    x8_ct.append(xc8)
                for dt in range(1, NDT):
                    nc.scalar.dma_start(wq_dt[dt][:], wq_d[:, ts(dt, FH)])
                    nc.scalar.dma_start(wk_dt[dt][:], wk_d[:, ts(dt, FH)])
                wq8_sb = proj_pool.tile([P, NDT, FH], FP8)
                wk8_sb = proj_pool.tile([P, NDT, FH], FP8)
                wv8_sb = proj_pool.tile([P, NDT, FH], FP8)
                nc.scalar.dma_start(wq8_sb[:], wq8_d[:].rearrange("p (dt f) -> p dt f", f=FH))
                nc.scalar.dma_start(wk8_sb[:], wk8_d[:].rearrange("p (dt f) -> p dt f", f=FH))
                nc.scalar.dma_start(wv8_sb[:], wv8_d[:].rearrange("p (dt f) -> p dt f", f=FH))
                nc.scalar.dma_start(wv_sb[:], wv_d[:].rearrange("p (dt f) -> p dt f", f=FH))
                nc.scalar.dma_start(wo_sb[:], wo_d[:].rearrange("p (ft e) -> p ft e", e=D))

                # ACT exp-table load well before the first attention exp
                warm = persist.tile([P, 16], F32)
                nc.vector.memset(warm[:], 0.0)
                nc.scalar.activation(warm[:], warm[:], AF.Exp)

                with (
                    tc.tile_pool(name="attn_sb", bufs=6) as ap_pool,
                    tc.tile_pool(name="psum_a", bufs=1, space=bass.MemorySpace.PSUM) as pa,
                    tc.tile_pool(name="norm", bufs=3) as norm_pool,
                    tc.tile_pool(name="out_sb", bufs=3) as ot_pool,
                ):
                    # ---- 128-mode work units, emitted either as a burst or
                    # threaded into the attention stream as PE filler ----

                    def qk_chunk_mms(c):
                        """Closures projecting q/k for column chunk c.

                        Chunk 0 runs bf16 (rows 0-511 have too little
                        softmax support to average away fp8 noise); later
                        chunks contract d-tile PAIRS in fp8 DoubleRow."""
                        mms = []
                        specs = (
                            ((wq_dt, wq8_sb), bq_sb, qT),
                            ((wk_dt, wk8_sb), bk_sb, kT),
                        )
                        for (w_dt, w8), b_sb, dst in specs:
                            for ft in range(NFT):
                                box = {}

                                def alloc(box=box, c=c, ft=ft):
                                    box["ps"] = pa.tile(
                                        [P, QC], F32, tag="fill", bufs=2,
                                        name=f"pq{c}_{ft}",
                                    )

                                if c in (0, NQC - 1):
                                    for dt in range(NDT):
                                        def mm(box=box, w_dt=w_dt, dt=dt, ft=ft, c=c):
                                            if dt == 0:
                                                box["alloc"]()
                                            nc.tensor.matmul(
                                                box["ps"][:],
                                                w_dt[dt][:, ts(ft, P)],
                                                x_ct[c][dt][:],
                                                start=(dt == 0),
                                                stop=(dt == NDT - 1),
                                            )
                                        box["alloc"] = alloc
                                        mms.append(mm)
                                else:
                                    for dp in range(NDT // 2):
                                        def mm(box=box, w8=w8, dp=dp, ft=ft, c=c):
                                            if dp == 0:
                                                box["alloc"]()
                                            nc.tensor.matmul(
                                                box["ps"][:],
                                                w8[:, 2 * dp : 2 * dp + 2, ts(ft, P)],
                                                x8_ct[c][:, 2 * dp : 2 * dp + 2, :],
                                                start=(dp == 0),
                                                stop=(dp == NDT // 2 - 1),
                                                perf_mode=mybir.MatmulPerfMode.DoubleRow,
                                            )
                                        box["alloc"] = alloc
                                        mms.append(mm)

                                def evac(box=box, b_sb=b_sb, dst=dst, ft=ft, c=c):
                                    nc.scalar.activation(
                                        dst[:, ft, ts(c, QC)],
                                        box["ps"][:],
                                        AF.Identity,
                                        bias=b_sb[:, ft : ft + 1],
                                    )
                                mms.append(evac)
                        return mms

                    def v_proj_mms(qc):
                        """Closures projecting v for chunk qc's k-tiles."""
                        mms = []
                        for kt in range(qc * (QC // KT), (qc + 1) * (QC // KT)):
                            box = {}

                            def alloc(box=box, kt=kt):
                                box["ps"] = pa.tile(
                                    [P, FH], F32, tag="fill", bufs=2, name=f"pv{kt}"
                                )

                            if qc in (0, NQC - 1):
                                for dt in range(NDT):
                                    def mm(box=box, kt=kt, dt=dt, qc=qc):
                                        if dt == 0:
                                            box["alloc"]()
                                        nc.tensor.matmul(
                                            box["ps"][:],
                                            x_ct[qc][dt][:, ts(kt % 4, KT)],
                                            wv_sb[:, dt, :],
                                            start=(dt == 0),
                                            stop=(dt == NDT - 1),
                                        )
                                    box["alloc"] = alloc
                                    mms.append(mm)
                            else:
                                for dp in range(NDT // 2):
                                    def mm(box=box, kt=kt, dp=dp, qc=qc):
                                        if dp == 0:
                                            box["alloc"]()
                                        nc.tensor.matmul(
                                            box["ps"][:],
                                            x8_ct[qc][:, 2 * dp : 2 * dp + 2, ts(kt % 4, KT)],
                                            wv8_sb[:, 2 * dp : 2 * dp + 2, :],
                                            start=(dp == 0),
                                            stop=(dp == NDT // 2 - 1),
                                            perf_mode=mybir.MatmulPerfMode.DoubleRow,
                                        )
                                    box["alloc"] = alloc
                                    mms.append(mm)

                            def evac(box=box, kt=kt):
                                nc.vector.tensor_copy(
                                    v_sb[:, kt, :, 0:DH],
                                    box["ps"][:].rearrange("p (h d) -> p h d", h=HG),
                                )
                                if kt < 2:
                                    nc.vector.tensor_copy(
                                        v_bf[:, kt, :, 0:DH],
                                        box["ps"][:].rearrange("p (h d) -> p h d", h=HG),
                                    )
                            mms.append(evac)
                        return mms

                    def out_proj_mms(qc, evac_on_act=False):
                        """Closures for a finished q-range's output projection."""
                        mms = []
                        for qb in range(qc * (QC // P), (qc + 1) * (QC // P)):
                            box = {}

                            def alloc(box=box, qb=qb):
                                box["ps"] = [
                                    pa.tile(
                                        [P, QC], F32, tag="fill", bufs=2,
                                        name=f"po{qb}_{eh}",
                                    )
                                    for eh in range(NEH)
                                ]

                            for eh in range(NEH):
                                for ft in range(NFT):
                                    def mm(box=box, qb=qb, eh=eh, ft=ft):
                                        if eh == 0 and ft == 0:
                                            box["alloc"]()
                                        nc.tensor.matmul(
                                            box["ps"][eh][:],
                                            aTn[:, ft, ts(qb, P)],
                                            wo_sb[:, ft, ts(eh, QC)],
                                            start=(ft == 0),
                                            stop=(ft == NFT - 1),
                                        )
                                    box["alloc"] = alloc
                                    mms.append(mm)

                            def evac(box=box, qb=qb):
                                ot = ot_pool.tile([P, D], OUTDT, tag="ot", name=f"ot{qb}")
                                for eh in range(NEH):
                                    if evac_on_act and eh == 0:
                                        nc.scalar.activation(
                                            ot[:, ts(eh, QC)], box["ps"][eh][:], AF.Identity
                                        )
                                    else:
                                        nc.vector.tensor_copy(ot[:, ts(eh, QC)], box["ps"][eh][:])
                                    if evac_on_act:
                                        nc.sync.dma_start(
                                            out_d[ts(qb, P), ts(eh, QC)], ot[:, ts(eh, QC)]
                                        )
                                if not evac_on_act:
                                    nc.sync.dma_start(out_d[ts(qb, P), :], ot[:])
                            mms.append(evac)
                        return mms

                    def emit(queue, n):
                        for _ in range(min(n, len(queue))):
                            queue.pop(0)()

                    wps = pa.tile([P, QC], F32, tag="fill", bufs=2, name="warmps")
                    for _ in range(16):
                        nc.tensor.matmul(
                            wps[:], warm_w[:], warm_x[:], start=True, stop=True
                        )

                    # chunk 0 + its v projection run as a plain burst (there
                    # is no attention to interleave with yet)
                    boot = qk_chunk_mms(0) + v_proj_mms(0)
                    emit(boot, len(boot))

                    qcs = list(range(NQC))
                    for qi, qc in enumerate(qcs):
                        nkt = (qc + 1) * (QC // KT)
                        # 128-mode filler for this chunk's attention stream:
                        # next q/k chunk, next v tiles, previous out columns
                        filler = []
                        if qi + 1 < NQC:
                            filler += qk_chunk_mms(qi + 1)
                            filler += v_proj_mms(qi + 1)
                        if qi == 1:
                            filler += out_proj_mms(qcs[0])
                        elif qi == NQC - 1:
                            filler += out_proj_mms(qcs[qi - 2])
                            filler += out_proj_mms(qcs[qi - 1])

                        for hp in range(NFT):
                            heads = (2 * hp, 2 * hp + 1)
                            psa = {
                                h: pa.tile([DH + 1, QC], F32, tag="psa", bufs=2, name=f"psa{h}_{qc}")
                                for h in heads
                            }
                            pending = []

                            def flush_pair(filler=filler):
                                # pop both heads' tiles of one k-tile pair
                                # and contract both k-tiles in ONE fp8
                                # DoubleRow matmul (2 weights per PE cell),
                                # then thread in some 128-mode filler
                                for _ in range(2):
                                    h_, pt_, cc_ = pending.pop(0)
                                    kt0_, _, c0a_ = cc_[0]
                                    if qc == 0 and kt0_ == 0:
                                        for u_, (kt_, t_, c0_) in enumerate(cc_):
                                            nc.tensor.matmul(
                                                psa[h_][:, ds(c0_, QC - c0_)],
                                                v_bf[:, kt_, h_, :],
                                                pt_[:, ds(u_ * QC + c0_, QC - c0_)],
                                                start=(kt_ == 0),
                                                stop=False,
                                            )
                                    else:
                                        pt3 = pt_[:].rearrange("p (u q) -> p u q", u=2)
                                        nc.tensor.matmul(
                                            psa[h_][:, ds(c0a_, QC - c0a_)],
                                            v_sb[:, kt0_ : kt0_ + 2, h_, 0 : DH + 1],
                                            pt3[:, :, c0a_:],
                                            start=(kt0_ == 0 and qc != 0),
                                            stop=(kt0_ == nkt - 2),
                                            perf_mode=mybir.MatmulPerfMode.DoubleRow,
                                        )
                                emit(filler, 4)

                            for ktp in range(0, nkt, 2):
                                cc = []
                                for u in (0, 1):
                                    kt = ktp + u
                                    t = kt - qc * (QC // KT)
                                    c0 = KT * t if t > 0 else 0
                                    cc.append((kt, t, c0))
                                tiles = {
                                    h: (
                                        pa.tile(
                                            [P, 2 * QC], F32, tag="pss", bufs=2,
                                            name=f"pss{h}_{qc}_{ktp}",
                                        ),
                                        ap_pool.tile(
                                            [P, 2 * QC],
                                            MMDT if (qc == 0 and ktp == 0) else PVDT,
                                            tag="pt", bufs=6,
                                            name=f"pt{h}_{qc}_{ktp}",
                                        ),
                                    )
                                    for h in heads
                                }
                                for u, (kt, t, c0) in enumerate(cc):
                                    for h in heads:
                                        pb = DH * (h % 2)
                                        pss, pt = tiles[h]
                                        nc.tensor.matmul(
                                            pss[:, ds(u * QC + c0, QC - c0)],
                                            kT[pb : pb + DH, hp, ts(kt, KT)],
                                            qT[pb : pb + DH, hp, ds(qc * QC + c0, QC - c0)],
                                            start=True,
                                            stop=True,
                                            tile_position=(pb, 0),
                                        )
                                for h in heads:
                                    pss, pt = tiles[h]
                                    c0a, c0b = cc[0][2], cc[1][2]
                                    if c0a == 0 and c0b == 0:
                                        nc.scalar.activation(pt[:], pss[:], AF.Exp)
                                        for u, (kt, t, c0) in enumerate(cc):
                                            if t >= 0:
                                                reg = pt[:, ds(u * QC + c0, KT)]
                                                nc.vector.tensor_mul(reg, reg, tri[:])
                                    elif qc == 0:
                                        for u, (kt, t, c0) in enumerate(cc):
                                            nc.scalar.activation(
                                                pt[:, ds(u * QC + c0, QC - c0)],
                                                pss[:, ds(u * QC + c0, QC - c0)],
                                                AF.Exp,
                                            )
                                            nc.vector.tensor_mul(
                                                pt[:, ds(u * QC + c0, KT)],
                                                pt[:, ds(u * QC + c0, KT)],
                                                tri_bf[:] if ktp == 0 else tri[:],
                                            )
                                        if ktp > 0 and c0b > c0a:
                                            nc.vector.memset(
                                                pt[:, ds(QC + c0a, c0b - c0a)], 0.0
                                            )
                                    else:
                                        # one exp covers both tiles from col
                                        # c0a; tile u1's stale+triangle region
                                        # is zeroed with the 256-wide mask
                                        v3 = pss[:].rearrange("p (u q) -> p u q", u=2)
                                        o3 = pt[:].rearrange("p (u q) -> p u q", u=2)
                                        nc.scalar.activation(
                                            o3[:, :, c0a:], v3[:, :, c0a:], AF.Exp
                                        )
                                        nc.vector.tensor_mul(
                                            pt[:, ds(c0a, KT)],
                                            pt[:, ds(c0a, KT)],
                                            tri[:],
                                        )
                                        nc.vector.tensor_mul(
                                            pt[:, ds(QC + c0a, 2 * KT)],
                                            pt[:, ds(QC + c0a, 2 * KT)],
                                            tri2[:],
                                        )
                                    pending.append((h, pt, cc))
                                while len(pending) > 4:
                                    flush_pair()
                            while pending:
                                flush_pair()

                            # ---- normalize (broadcast only ever writes
                            # base-partition-0 tiles); the last f-tile runs
                            # the whole chain per column half so the tail
                            # out-projection starts earlier ----
                            nt = {}
                            for h in heads:
                                nt[h] = (
                                    norm_pool.tile([DH, QC], F32, tag="araw", bufs=4, name=f"araw{h}_{qc}"),
                                    norm_pool.tile([1, QC], F32, tag="se", bufs=4, name=f"se{h}_{qc}"),
                                    norm_pool.tile([DH, QC], F32, tag="sebc", bufs=4, name=f"sebc{h}_{qc}"),
                                    norm_pool.tile([DH, QC], F32, tag="rec", bufs=4, name=f"rec{h}_{qc}"),
                                )
                            last_hp = qc == NQC - 1 and hp == NFT - 1
                            halves = ((0, QC // 2), (QC // 2, QC // 2)) if last_hp else ((0, QC),)
                            for off, w in halves:
                                sl = ds(off, w)
                                for h in heads:
                                    if qc == NQC - 1:
                                        nc.scalar.activation(
                                            nt[h][1][:, sl], psa[h][DH : DH + 1, sl], AF.Identity
                                        )
                                        nc.scalar.activation(
                                            nt[h][0][:, sl], psa[h][0:DH, sl], AF.Identity
                                        )
                                    else:
                                        nc.vector.tensor_copy(nt[h][1][:, sl], psa[h][DH : DH + 1, sl])
                                        nc.vector.tensor_copy(nt[h][0][:, sl], psa[h][0:DH, sl])
                                for h in heads:
                                    nc.gpsimd.partition_broadcast(nt[h][2][:, sl], nt[h][1][:, sl])
                                for h in heads:
                                    nc.vector.reciprocal_approx_fast(nt[h][3][:, sl], nt[h][2][:, sl])
                                for h in heads:
                                    pb = DH * (h % 2)
                                    nc.vector.tensor_mul(
                                        aTn[pb : pb + DH, hp, ds(qc * QC + off, w)],
                                        nt[h][0][:, sl],
                                        nt[h][3][:, sl],
                                    )
                        # drain leftover filler as a burst before the next
                        # q-chunk's attention
                        emit(filler, len(filler))

                    tail = out_proj_mms(qcs[-1], evac_on_act=True)
                    emit(tail, len(tail))

    nc.finalize()
    return nc


_NC_CACHE = {}


def get_program():
    if "nc" not in _NC_CACHE:
        _NC_CACHE["nc"] = build_program()
    return _NC_CACHE["nc"]


def _x_img(xb):
    """x [S, D] -> chunk-major image [P, (chunk, dt, qcols)]."""
    a = to_mmdt(xb.T)                       # [D, S]
    a = a.reshape(NDT, P, NQC, QC).transpose(1, 2, 0, 3)
    return np.ascontiguousarray(a.reshape(P, NDT * S))


def _x8_img(xb):
    """x [S, D] -> chunk-major fp8 image [P, (chunk, dt, qcols)]."""
    a = to_fp8(xb.T)
    a = a.reshape(NDT, P, NQC, QC).transpose(1, 2, 0, 3)
    return np.ascontiguousarray(a.reshape(P, NDT * S))


def _img8(a):
    """[NDT*P, F] -> partition-major fp8 image [P, NDT*F]."""
    a = to_fp8(a)
    return np.ascontiguousarray(
        a.reshape(NDT, P, FH).transpose(1, 0, 2).reshape(P, NDT * FH)
    )


def _img(a, nt):
    """[nt*P, F] -> partition-major SBUF image [P, nt*F]."""
    ntp, f = a.shape
    assert ntp == nt * P
    return np.ascontiguousarray(
        a.reshape(nt, P, f).transpose(1, 0, 2).reshape(P, nt * f)
    )


def shard_inputs(x, mask, Wq, bq, Wk, bk, Wv, bv, Wo, bo):
    del mask  # causality is structural in the kernel
    in_maps = []
    for c in range(N_CORES):
        b = c // 4
        g = c % 4
        fsl = slice(FH * g, FH * (g + 1))
        in_maps.append(
            {
                "x_img": _x_img(x[b]),
                "x8_img": _x8_img(x[b]),
                "wq8_img": _img8(Wq[fsl, :].T / 8.0),
                "wk8_img": _img8(Wk[fsl, :].T),
                "wv8_img": _img8(Wv[fsl, :].T),
                "wq_img": _img(to_mmdt(Wq[fsl, :].T / 8.0), NDT),
                "wk_img": _img(to_mmdt(Wk[fsl, :].T), NDT),
                "wv_img": _img(to_mmdt(Wv[fsl, :].T), NDT),
                "wo_img": _img(to_mmdt(Wo[:, fsl].T), NFT),
                "bq2": np.ascontiguousarray(
                    (bq[fsl] / 8.0).reshape(NFT, P).T.astype(np.float32)
                ),
                "bk2": np.ascontiguousarray(
                    bk[fsl].reshape(NFT, P).T.astype(np.float32)
                ),
            }
        )
    return in_maps


def gather_outputs(results, bias_term):
    out = np.zeros((B, S, D), dtype=np.float32)
    for b in range(B):
        acc = results[4 * b]["out"].astype(np.float32)
        for g in range(1, 4):
            acc = acc + results[4 * b + g]["out"].astype(np.float32)
        out[b] = acc + bias_term
    return out


def kernel(x, mask, Wq, bq, Wk, bk, Wv, bv, Wo, bo, **run_kwargs):
    x = np.asarray(x)
    mask = np.asarray(mask)
    Wq, bq = np.asarray(Wq), np.asarray(bq)
    Wk, bk = np.asarray(Wk), np.asarray(bk)
    Wv, bv = np.asarray(Wv), np.asarray(bv)
    Wo, bo = np.asarray(Wo), np.asarray(bo)

    nc = get_program()
    in_maps = shard_inputs(x, mask, Wq, bq, Wk, bk, Wv, bv, Wo, bo)
    res = run_bass_kernel_spmd(nc, in_maps, core_ids=list(range(N_CORES)), **run_kwargs)
    # bias term that commutes with the cross-core reduction
    bias_term = (bo.astype(np.float32) + Wo.astype(np.float32) @ bv.astype(np.float32))
    out = gather_outputs(res.results, bias_term)
    kernel.last_results = res
    return out


# revision 40
# speedup vs baseline: 1.0013x; 1.0013x over previous
"""Causal multi-head attention on 8 Trainium2 NeuronCores.

Sharding: data-parallel over batch (B=2) x tensor-parallel over heads
(16 heads -> 4 groups of 4). Core c handles batch c//4, heads
[4*(c%4), 4*(c%4)+4). Each core computes its head-slice QKV projections,
causal softmax attention, and a partial output projection (row-sharded
Wo). The host sums the 4 partials per batch and adds the biases that
commute with the reduction (bo + Wo @ bv).

Schedule (v3, ~157us vs 173us baseline):
  - x is DMA'd q-chunk-major (one 8KB-run DMA per chunk, split over 4
    queues) and Q/K are projected one 512-column chunk at a time, so
    attention for chunk 0 starts ~8us in instead of ~30us.
  - the attention inner loop is ACT(exp)-bound; the remaining Q/K
    chunks, the next chunk's v projection and the previous chunk's
    output projection are threaded INTO the attention stream as
    128-mode "filler" matmuls after each p@v burst, keeping the PE
    busy so the HAM clock gate never re-throttles (idle-heavy
    schedules measured 13-24us stretches at 1.2GHz).
  - p@v runs in fp8e4 DoubleRow: one matmul contracts a k-tile PAIR
    (2 weights per PE cell), halving p@v streaming time. exp writes
    pt straight to fp8. The first k-tile pair of chunk 0 stays bf16:
    rows with tiny softmax support (q<256) cannot average away fp8
    noise (measured 3e-2 rel err at row 0 with all-fp8, 1/sqrt(n)
    decay).
  - Q/K/V projections for chunks 1-2 also run fp8 DoubleRow (d-tile
    pairs, K=256/pass). Chunks 0 and 3 stay bf16: chunk 0 for the
    same small-support precision reason, chunk 3 deliberately, as
    cheap PE filler in the ACT-bound tail (thin filler there lets
    the HAM clock gate re-throttle the PE to 1.2GHz).
  - the last chunk's normalize copies and output evacuations run on
    the Scalar engine (idle at the tail) instead of the congested
    DVE queue.
  - diagonal-pair exps are merged into one 3D-AP activation; the
    second tile's stale+triangle region is zeroed with a 256-wide
    [zeros|triangle] mask.
  - the last f-tile's normalize multiply is split per 128 columns so
    the tail output projection starts per-block.
  - partial output is shipped bf16 (halves the out DMA).

Hardware notes baked into this design (all measured on-device):
  - 64x128 row-tiled T0/T8 matmul pairs DO run concurrently
    (108ns/MM) but only in homogeneous instruction streams; tiling-
    mode switches cost ~105ns PE drain each, so 64-mode scores and
    128-mode everything-else are batched.
  - gpsimd partition_broadcast silently fails for output base
    partition 64 (only base-0 tiles work).
  - DVE tensor_tensor cannot read two PSUM operands (one PSUM read
    port) and cannot shift partitions (tensor_copy can).
"""

import os

os.environ.setdefault("MYCRO_LOCAL_CACHE", "1")

import ml_dtypes
import numpy as np

import concourse.bass as bass
import concourse.tile as tile
from concourse import bacc, mybir
from concourse.bass import ds, ts
from concourse.bass_utils import run_bass_kernel_spmd

AF = mybir.ActivationFunctionType

B = 2
S = 2048
D = 1024
N_HEADS = 16
DH = 64
N_CORES = 8

HG = 4            # heads per core
FH = HG * DH      # 256 features per core
P = 128
NFT = FH // P     # 2 f-tiles per core
NDT = D // P      # 8 d_model tiles
QC = 512          # q chunk (moving free dim)
NQC = S // QC     # 4
KT = 128          # k tile (partition dim of sT)
NKT = S // KT     # 16
NEH = D // QC     # 2 output-projection column halves

F32 = mybir.dt.float32
BF16 = mybir.dt.bfloat16
FP8 = mybir.dt.float8e4

MMDT = BF16
PVDT = FP8        # p@v operand dtype (DoubleRow: 2 k-tiles per matmul)
OUTDT = BF16      # partial-output dtype shipped to host
VPAD = 68         # v row padded so the DoubleRow weight step is 16B-aligned


def to_mmdt(a):
    a = np.ascontiguousarray(np.asarray(a, np.float32))
    return np.ascontiguousarray(a.astype(ml_dtypes.bfloat16))


def to_fp8(a):
    a = np.ascontiguousarray(np.asarray(a, np.float32))
    return np.ascontiguousarray(a.astype(ml_dtypes.float8_e4m3fn))


def build_program():
    nc = bacc.Bacc(None, target_bir_lowering=False)

    x_d = nc.dram_tensor("x_img", [P, NDT * S], MMDT, kind="ExternalInput")
    wq_d = nc.dram_tensor("wq_img", [P, NDT * FH], MMDT, kind="ExternalInput")
    wk_d = nc.dram_tensor("wk_img", [P, NDT * FH], MMDT, kind="ExternalInput")
    wv_d = nc.dram_tensor("wv_img", [P, NDT * FH], MMDT, kind="ExternalInput")
    wo_d = nc.dram_tensor("wo_img", [P, NFT * D], MMDT, kind="ExternalInput")
    x8_d = nc.dram_tensor("x8_img", [P, NDT * S], FP8, kind="ExternalInput")
    wq8_d = nc.dram_tensor("wq8_img", [P, NDT * FH], FP8, kind="ExternalInput")
    wk8_d = nc.dram_tensor("wk8_img", [P, NDT * FH], FP8, kind="ExternalInput")
    wv8_d = nc.dram_tensor("wv8_img", [P, NDT * FH], FP8, kind="ExternalInput")
    bq_d = nc.dram_tensor("bq2", [P, NFT], F32, kind="ExternalInput")
    bk_d = nc.dram_tensor("bk2", [P, NFT], F32, kind="ExternalInput")
    out_d = nc.dram_tensor("out", [S, D], OUTDT, kind="ExternalOutput")

    with tile.TileContext(nc) as tc:
        with tc.tile_pool(name="persist", bufs=1) as persist:
            qT = persist.tile([P, NFT, S], MMDT)
            kT = persist.tile([P, NFT, S], MMDT)
            v_sb = persist.tile([P, NKT, HG, VPAD], PVDT)
            v_bf = persist.tile([P, 2, HG, DH + 1], MMDT)
            aTn = persist.tile([P, NFT, S], MMDT)
            wo_sb = persist.tile([P, NFT, D], MMDT)
            bq_sb = persist.tile([P, NFT], F32)
            bk_sb = persist.tile([P, NFT], F32)

            nc.scalar.dma_start(bq_sb[:], bq_d[:])
            nc.scalar.dma_start(bk_sb[:], bk_d[:])
            nc.vector.memset(v_sb[:, :, :, DH : DH + 1], 1.0)
            nc.vector.memset(v_bf[:, :, :, DH : DH + 1], 1.0)

            # triangle mask (keep k<=q) for the causal diagonal, and a
            # [zeros | triangle] double-width variant for the merged
            # diagonal-pair exp (zeroes the stale region + the triangle)
            tri = persist.tile([P, KT], PVDT)
            tri_bf = persist.tile([P, KT], MMDT)
            nc.vector.memset(tri[:], 1.0)
            nc.vector.memset(tri_bf[:], 1.0)
            nc.gpsimd.affine_select(
                out=tri_bf[:],
                in_=tri_bf[:],
                compare_op=mybir.AluOpType.is_ge,
                fill=0.0,
                base=0,
                channel_multiplier=-1,
                pattern=[[1, KT]],
            )
            nc.gpsimd.affine_select(
                out=tri[:],
                in_=tri[:],
                compare_op=mybir.AluOpType.is_ge,
                fill=0.0,
                base=0,
                channel_multiplier=-1,
                pattern=[[1, KT]],
            )
            warm_w = persist.tile([P, KT], MMDT)
            warm_x = persist.tile([P, QC], MMDT)
            nc.vector.memset(warm_w[:], 0.25)
            nc.vector.memset(warm_x[:], 0.25)
            tri2 = persist.tile([P, 2 * KT], PVDT)
            nc.vector.memset(tri2[:], 1.0)
            nc.gpsimd.affine_select(
                out=tri2[:],
                in_=tri2[:],
                compare_op=mybir.AluOpType.is_ge,
                fill=0.0,
                base=-KT,
                channel_multiplier=-1,
                pattern=[[1, 2 * KT]],
            )
            with tc.tile_pool(name="proj", bufs=1) as proj_pool:
                # per-dt weight tiles and chunk-major x tiles: the first
                # projection matmul waits on 64KB of weights + 128KB of x
                wq_dt = [proj_pool.tile([P, FH], MMDT, name=f"wq{dt}") for dt in range(NDT)]
                wk_dt = [proj_pool.tile([P, FH], MMDT, name=f"wk{dt}") for dt in range(NDT)]
                wv_sb = proj_pool.tile([P, NDT, FH], MMDT)
                nc.scalar.dma_start(wq_dt[0][:], wq_d[:, ts(0, FH)])
                nc.scalar.dma_start(wk_dt[0][:], wk_d[:, ts(0, FH)])
                x0 = proj_pool.tile([P, NDT, QC], MMDT, name="x0")
                for part in range(4):
                    nc.sync.dma_start(
                        x0[:, 2 * part : 2 * part + 2, :],
                        x_d[:, ds(2 * part * QC, 2 * QC)].rearrange(
                            "p (dt q) -> p dt q", q=QC
                        ),
                    )
                x3 = proj_pool.tile([P, NDT, QC], MMDT, name="x3")
                for part in range(4):
                    nc.sync.dma_start(
                        x3[:, 2 * part : 2 * part + 2, :],
                        x_d[:, ds((3 * NDT + 2 * part) * QC, 2 * QC)].rearrange(
                            "p (dt q) -> p dt q", q=QC
                        ),
                    )
                xbf = {0: x0, 3: x3}
                x_ct = {c: [xbf[c][:, dt, :] for dt in range(NDT)] for c in (0, 3)}
                x8_ct = [None]
                for c in (1, 2):
                    xc8 = proj_pool.tile([P, NDT, QC], FP8, name=f"x8_{c}")
                    for part in range(2):
                        nc.sync.dma_start(
                            xc8[:, 4 * part : 4 * part + 4, :],
                            x8_d[:, ds((c * NDT + 4 * part) * QC, 4 * QC)].rearrange(
                                "p (dt q) -> p dt q", q=QC
                            ),
                        )
                    x8_ct.append(xc8)
                for dt in range(1, NDT):
                    nc.scalar.dma_start(wq_dt[dt][:], wq_d[:, ts(dt, FH)])
                    nc.scalar.dma_start(wk_dt[dt][:], wk_d[:, ts(dt, FH)])
                wq8_sb = proj_pool.tile([P, NDT, FH], FP8)
                wk8_sb = proj_pool.tile([P, NDT, FH], FP8)
                wv8_sb = proj_pool.tile([P, NDT, FH], FP8)
                nc.scalar.dma_start(wq8_sb[:], wq8_d[:].rearrange("p (dt f) -> p dt f", f=FH))
                nc.scalar.dma_start(wk8_sb[:], wk8_d[:].rearrange("p (dt f) -> p dt f", f=FH))
                nc.scalar.dma_start(wv8_sb[:], wv8_d[:].rearrange("p (dt f) -> p dt f", f=FH))
                nc.scalar.dma_start(wv_sb[:], wv_d[:].rearrange("p (dt f) -> p dt f", f=FH))
                nc.scalar.dma_start(wo_sb[:], wo_d[:].rearrange("p (ft e) -> p ft e", e=D))

                # ACT exp-table load well before the first attention exp
                warm = persist.tile([P, 16], F32)
                nc.vector.memset(warm[:], 0.0)
                nc.scalar.activation(warm[:], warm[:], AF.Exp)

                with (
                    tc.tile_pool(name="attn_sb", bufs=6) as ap_pool,
                    tc.tile_pool(name="psum_a", bufs=1, space=bass.MemorySpace.PSUM) as pa,
                    tc.tile_pool(name="norm", bufs=3) as norm_pool,
                    tc.tile_pool(name="out_sb", bufs=3) as ot_pool,
                ):
                    # ---- 128-mode work units, emitted either as a burst or
                    # threaded into the attention stream as PE filler ----

                    def qk_chunk_mms(c):
                        """Closures projecting q/k for column chunk c.

                        Chunk 0 runs bf16 (rows 0-511 have too little
                        softmax support to average away fp8 noise); later
                        chunks contract d-tile PAIRS in fp8 DoubleRow."""
                        mms = []
                        specs = (
                            ((wq_dt, wq8_sb), bq_sb, qT),
                            ((wk_dt, wk8_sb), bk_sb, kT),
                        )
                        for (w_dt, w8), b_sb, dst in specs:
                            for ft in range(NFT):
                                box = {}

                                def alloc(box=box, c=c, ft=ft):
                                    box["ps"] = pa.tile(
                                        [P, QC], F32, tag="fill", bufs=2,
                                        name=f"pq{c}_{ft}",
                                    )

                                if c in (0, NQC - 1):
                                    for dt in range(NDT):
                                        def mm(box=box, w_dt=w_dt, dt=dt, ft=ft, c=c):
                                            if dt == 0:
                                                box["alloc"]()
                                            nc.tensor.matmul(
                                                box["ps"][:],
                                                w_dt[dt][:, ts(ft, P)],
                                                x_ct[c][dt][:],
                                                start=(dt == 0),
                                                stop=(dt == NDT - 1),
                                            )
                                        box["alloc"] = alloc
                                        mms.append(mm)
                                else:
                                    for dp in range(NDT // 2):
                                        def mm(box=box, w8=w8, dp=dp, ft=ft, c=c):
                                            if dp == 0:
                                                box["alloc"]()
                                            nc.tensor.matmul(
                                                box["ps"][:],
                                                w8[:, 2 * dp : 2 * dp + 2, ts(ft, P)],
                                                x8_ct[c][:, 2 * dp : 2 * dp + 2, :],
                                                start=(dp == 0),
                                                stop=(dp == NDT // 2 - 1),
                                                perf_mode=mybir.MatmulPerfMode.DoubleRow,
                                            )
                                        box["alloc"] = alloc
                                        mms.append(mm)

                                def evac(box=box, b_sb=b_sb, dst=dst, ft=ft, c=c):
                                    nc.scalar.activation(
                                        dst[:, ft, ts(c, QC)],
                                        box["ps"][:],
                                        AF.Identity,
                                        bias=b_sb[:, ft : ft + 1],
                                    )
                                mms.append(evac)
                        return mms

                    def v_proj_mms(qc):
                        """Closures projecting v for chunk qc's k-tiles."""
                        mms = []
                        for kt in range(qc * (QC // KT), (qc + 1) * (QC // KT)):
                            box = {}

                            def alloc(box=box, kt=kt):
                                box["ps"] = pa.tile(
                                    [P, FH], F32, tag="fill", bufs=2, name=f"pv{kt}"
                                )

                            if qc in (0, NQC - 1):
                                for dt in range(NDT):
                                    def mm(box=box, kt=kt, dt=dt, qc=qc):
                                        if dt == 0:
                                            box["alloc"]()
                                        nc.tensor.matmul(
                                            box["ps"][:],
                                            x_ct[qc][dt][:, ts(kt % 4, KT)],
                                            wv_sb[:, dt, :],
                                            start=(dt == 0),
                                            stop=(dt == NDT - 1),
                                        )
                                    box["alloc"] = alloc
                                    mms.append(mm)
                            else:
                                for dp in range(NDT // 2):
                                    def mm(box=box, kt=kt, dp=dp, qc=qc):
                                        if dp == 0:
                                            box["alloc"]()
                                        nc.tensor.matmul(
                                            box["ps"][:],
                                            x8_ct[qc][:, 2 * dp : 2 * dp + 2, ts(kt % 4, KT)],
                                            wv8_sb[:, 2 * dp : 2 * dp + 2, :],
                                            start=(dp == 0),
                                            stop=(dp == NDT // 2 - 1),
                                            perf_mode=mybir.MatmulPerfMode.DoubleRow,
                                        )
                                    box["alloc"] = alloc
                                    mms.append(mm)

                            def evac(box=box, kt=kt):
                                nc.vector.tensor_copy(
                                    v_sb[:, kt, :, 0:DH],
                                    box["ps"][:].rearrange("p (h d) -> p h d", h=HG),
                                )
                                if kt < 2:
                                    nc.vector.tensor_copy(
                                        v_bf[:, kt, :, 0:DH],
                                        box["ps"][:].rearrange("p (h d) -> p h d", h=HG),
                                    )
                            mms.append(evac)
                        return mms

                    def out_proj_mms(qc, evac_on_act=False):
                        """Closures for a finished q-range's output projection."""
                        mms = []
                        for qb in range(qc * (QC // P), (qc + 1) * (QC // P)):
                            box = {}

                            def alloc(box=box, qb=qb):
                                box["ps"] = [
                                    pa.tile(
                                        [P, QC], F32, tag="fill", bufs=2,
                                        name=f"po{qb}_{eh}",
                                    )
                                    for eh in range(NEH)
                                ]

                            for eh in range(NEH):
                                for ft in range(NFT):
                                    def mm(box=box, qb=qb, eh=eh, ft=ft):
                                        if eh == 0 and ft == 0:
                                            box["alloc"]()
                                        nc.tensor.matmul(
                                            box["ps"][eh][:],
                                            aTn[:, ft, ts(qb, P)],
                                            wo_sb[:, ft, ts(eh, QC)],
                                            start=(ft == 0),
                                            stop=(ft == NFT - 1),
                                        )
                                    box["alloc"] = alloc
                                    mms.append(mm)

                            def evac(box=box, qb=qb):
                                ot = ot_pool.tile([P, D], OUTDT, tag="ot", name=f"ot{qb}")
                                for eh in range(NEH):
                                    if evac_on_act and eh == 0:
                                        nc.scalar.activation(
                                            ot[:, ts(eh, QC)], box["ps"][eh][:], AF.Identity
                                        )
                                    else:
                                        nc.vector.tensor_copy(ot[:, ts(eh, QC)], box["ps"][eh][:])
                                    if evac_on_act:
                                        nc.sync.dma_start(
                                            out_d[ts(qb, P), ts(eh, QC)], ot[:, ts(eh, QC)]
                                        )
                                if not evac_on_act:
                                    nc.sync.dma_start(out_d[ts(qb, P), :], ot[:])
                            mms.append(evac)
                        return mms

                    def emit(queue, n):
                        for _ in range(min(n, len(queue))):
                            queue.pop(0)()

                    wps = pa.tile([P, QC], F32, tag="fill", bufs=2, name="warmps")
                    for _ in range(36):
                        nc.tensor.matmul(
                            wps[:], warm_w[:], warm_x[:], start=True, stop=True
                        )

                    # chunk 0 + its v projection run as a plain burst (there
                    # is no attention to interleave with yet)
                    boot = qk_chunk_mms(0) + v_proj_mms(0)
                    emit(boot, len(boot))

                    qcs = list(range(NQC))
                    for qi, qc in enumerate(qcs):
                        nkt = (qc + 1) * (QC // KT)
                        # 128-mode filler for this chunk's attention stream:
                        # next q/k chunk, next v tiles, previous out columns
                        filler = []
                        if qi + 1 < NQC:
                            filler += qk_chunk_mms(qi + 1)
                            filler += v_proj_mms(qi + 1)
                        if qi == 1:
                            filler += out_proj_mms(qcs[0])
                        elif qi == NQC - 1:
                            filler += out_proj_mms(qcs[qi - 2])
                            filler += out_proj_mms(qcs[qi - 1])

                        for hp in range(NFT):
                            heads = (2 * hp, 2 * hp + 1)
                            psa = {
                                h: pa.tile([DH + 1, QC], F32, tag="psa", bufs=2, name=f"psa{h}_{qc}")
                                for h in heads
                            }
                            pending = []

                            def flush_pair(filler=filler):
                                # pop both heads' tiles of one k-tile pair
                                # and contract both k-tiles in ONE fp8
                                # DoubleRow matmul (2 weights per PE cell),
                                # then thread in some 128-mode filler
                                for _ in range(2):
                                    h_, pt_, cc_ = pending.pop(0)
                                    kt0_, _, c0a_ = cc_[0]
                                    if qc == 0 and kt0_ == 0:
                                        for u_, (kt_, t_, c0_) in enumerate(cc_):
                                            nc.tensor.matmul(
                                                psa[h_][:, ds(c0_, QC - c0_)],
                                                v_bf[:, kt_, h_, :],
                                                pt_[:, ds(u_ * QC + c0_, QC - c0_)],
                                                start=(kt_ == 0),
                                                stop=False,
                                            )
                                    else:
                                        pt3 = pt_[:].rearrange("p (u q) -> p u q", u=2)
                                        nc.tensor.matmul(
                                            psa[h_][:, ds(c0a_, QC - c0a_)],
                                            v_sb[:, kt0_ : kt0_ + 2, h_, 0 : DH + 1],
                                            pt3[:, :, c0a_:],
                                            start=(kt0_ == 0 and qc != 0),
                                            stop=(kt0_ == nkt - 2),
                                            perf_mode=mybir.MatmulPerfMode.DoubleRow,
                                        )
                                emit(filler, 4)

                            for ktp in range(0, nkt, 2):
                                cc = []
                                for u in (0, 1):
                                    kt = ktp + u
                                    t = kt - qc * (QC // KT)
                                    c0 = KT * t if t > 0 else 0
                                    cc.append((kt, t, c0))
                                tiles = {
                                    h: (
                                        pa.tile(
                                            [P, 2 * QC], F32, tag="pss", bufs=2,
                                            name=f"pss{h}_{qc}_{ktp}",
                                        ),
                                        ap_pool.tile(
                                            [P, 2 * QC],
                                            MMDT if (qc == 0 and ktp == 0) else PVDT,
                                            tag="pt", bufs=6,
                                            name=f"pt{h}_{qc}_{ktp}",
                                        ),
                                    )
                                    for h in heads
                                }
                                for u, (kt, t, c0) in enumerate(cc):
                                    for h in heads:
                                        pb = DH * (h % 2)
                                        pss, pt = tiles[h]
                                        nc.tensor.matmul(
                                            pss[:, ds(u * QC + c0, QC - c0)],
                                            kT[pb : pb + DH, hp, ts(kt, KT)],
                                            qT[pb : pb + DH, hp, ds(qc * QC + c0, QC - c0)],
                                            start=True,
                                            stop=True,
                                            tile_position=(pb, 0),
                                        )
                                for h in heads:
                                    pss, pt = tiles[h]
                                    c0a, c0b = cc[0][2], cc[1][2]
                                    if c0a == 0 and c0b == 0:
                                        nc.scalar.activation(pt[:], pss[:], AF.Exp)
                                        for u, (kt, t, c0) in enumerate(cc):
                                            if t >= 0:
                                                reg = pt[:, ds(u * QC + c0, KT)]
                                                nc.vector.tensor_mul(reg, reg, tri[:])
                                    elif qc == 0:
                                        for u, (kt, t, c0) in enumerate(cc):
                                            nc.scalar.activation(
                                                pt[:, ds(u * QC + c0, QC - c0)],
                                                pss[:, ds(u * QC + c0, QC - c0)],
                                                AF.Exp,
                                            )
                                            nc.vector.tensor_mul(
                                                pt[:, ds(u * QC + c0, KT)],
                                                pt[:, ds(u * QC + c0, KT)],
                                                tri_bf[:] if ktp == 0 else tri[:],
                                            )
                                        if ktp > 0 and c0b > c0a:
                                            nc.vector.memset(
                                                pt[:, ds(QC + c0a, c0b - c0a)], 0.0
                                            )
                                    else:
                                        # one exp covers both tiles from col
                                        # c0a; tile u1's stale+triangle region
                                        # is zeroed with the 256-wide mask
                                        v3 = pss[:].rearrange("p (u q) -> p u q", u=2)
                                        o3 = pt[:].rearrange("p (u q) -> p u q", u=2)
                                        nc.scalar.activation(
                                            o3[:, :, c0a:], v3[:, :, c0a:], AF.Exp
                                        )
                                        nc.vector.tensor_mul(
                                            pt[:, ds(c0a, KT)],
                                            pt[:, ds(c0a, KT)],
                                            tri[:],
                                        )
                                        nc.vector.tensor_mul(
                                            pt[:, ds(QC + c0a, 2 * KT)],
                                            pt[:, ds(QC + c0a, 2 * KT)],
                                            tri2[:],
                                        )
                                    pending.append((h, pt, cc))
                                while len(pending) > 4:
                                    flush_pair()
                            while pending:
                                flush_pair()

                            # ---- normalize (broadcast only ever writes
                            # base-partition-0 tiles); the last f-tile runs
                            # the whole chain per column half so the tail
                            # out-projection starts earlier ----
                            nt = {}
                            for h in heads:
                                nt[h] = (
                                    norm_pool.tile([DH, QC], F32, tag="araw", bufs=4, name=f"araw{h}_{qc}"),
                                    norm_pool.tile([1, QC], F32, tag="se", bufs=4, name=f"se{h}_{qc}"),
                                    norm_pool.tile([DH, QC], F32, tag="sebc", bufs=4, name=f"sebc{h}_{qc}"),
                                    norm_pool.tile([DH, QC], F32, tag="rec", bufs=4, name=f"rec{h}_{qc}"),
                                )
                            last_hp = qc == NQC - 1 and hp == NFT - 1
                            halves = ((0, QC // 2), (QC // 2, QC // 2)) if last_hp else ((0, QC),)
                            for off, w in halves:
                                sl = ds(off, w)
                                for h in heads:
                                    if qc == NQC - 1:
                                        nc.scalar.activation(
                                            nt[h][1][:, sl], psa[h][DH : DH + 1, sl], AF.Identity
                                        )
                                        nc.scalar.activation(
                                            nt[h][0][:, sl], psa[h][0:DH, sl], AF.Identity
                                        )
                                    else:
                                        nc.vector.tensor_copy(nt[h][1][:, sl], psa[h][DH : DH + 1, sl])
                                        nc.vector.tensor_copy(nt[h][0][:, sl], psa[h][0:DH, sl])
                                for h in heads:
                                    nc.gpsimd.partition_broadcast(nt[h][2][:, sl], nt[h][1][:, sl])
                                for h in heads:
                                    nc.vector.reciprocal_approx_fast(nt[h][3][:, sl], nt[h][2][:, sl])
                                for h in heads:
                                    pb = DH * (h % 2)
                                    nc.vector.tensor_mul(
                                        aTn[pb : pb + DH, hp, ds(qc * QC + off, w)],
                                        nt[h][0][:, sl],
                                        nt[h][3][:, sl],
                                    )
                        # drain leftover filler as a burst before the next
                        # q-chunk's attention
                        emit(filler, len(filler))

                    tail = out_proj_mms(qcs[-1], evac_on_act=True)
                    emit(tail, len(tail))

    nc.finalize()
    return nc


_NC_CACHE = {}


def get_program():
    if "nc" not in _NC_CACHE:
        _NC_CACHE["nc"] = build_program()
    return _NC_CACHE["nc"]


def _x_img(xb):
    """x [S, D] -> chunk-major image [P, (chunk, dt, qcols)]."""
    a = to_mmdt(xb.T)                       # [D, S]
    a = a.reshape(NDT, P, NQC, QC).transpose(1, 2, 0, 3)
    return np.ascontiguousarray(a.reshape(P, NDT * S))


def _x8_img(xb):
    """x [S, D] -> chunk-major fp8 image [P, (chunk, dt, qcols)]."""
    a = to_fp8(xb.T)
    a = a.reshape(NDT, P, NQC, QC).transpose(1, 2, 0, 3)
    return np.ascontiguousarray(a.reshape(P, NDT * S))


def _img8(a):
    """[NDT*P, F] -> partition-major fp8 image [P, NDT*F]."""
    a = to_fp8(a)
    return np.ascontiguousarray(
        a.reshape(NDT, P, FH).transpose(1, 0, 2).reshape(P, NDT * FH)
    )


def _img(a, nt):
    """[nt*P, F] -> partition-major SBUF image [P, nt*F]."""
    ntp, f = a.shape
    assert ntp == nt * P
    return np.ascontiguousarray(
        a.reshape(nt, P, f).transpose(1, 0, 2).reshape(P, nt * f)
    )


def shard_inputs(x, mask, Wq, bq, Wk, bk, Wv, bv, Wo, bo):
    del mask  # causality is structural in the kernel
    in_maps = []
    for c in range(N_CORES):
        b = c // 4
        g = c % 4
        fsl = slice(FH * g, FH * (g + 1))
        in_maps.append(
            {
                "x_img": _x_img(x[b]),
                "x8_img": _x8_img(x[b]),
                "wq8_img": _img8(Wq[fsl, :].T / 8.0),
                "wk8_img": _img8(Wk[fsl, :].T),
                "wv8_img": _img8(Wv[fsl, :].T),
                "wq_img": _img(to_mmdt(Wq[fsl, :].T / 8.0), NDT),
                "wk_img": _img(to_mmdt(Wk[fsl, :].T), NDT),
                "wv_img": _img(to_mmdt(Wv[fsl, :].T), NDT),
                "wo_img": _img(to_mmdt(Wo[:, fsl].T), NFT),
                "bq2": np.ascontiguousarray(
                    (bq[fsl] / 8.0).reshape(NFT, P).T.astype(np.float32)
                ),
                "bk2": np.ascontiguousarray(
                    bk[fsl].reshape(NFT, P).T.astype(np.float32)
                ),
            }
        )
    return in_maps


def gather_outputs(results, bias_term):
    out = np.zeros((B, S, D), dtype=np.float32)
    for b in range(B):
        acc = results[4 * b]["out"].astype(np.float32)
        for g in range(1, 4):
            acc = acc + results[4 * b + g]["out"].astype(np.float32)
        out[b] = acc + bias_term
    return out


def kernel(x, mask, Wq, bq, Wk, bk, Wv, bv, Wo, bo, **run_kwargs):
    x = np.asarray(x)
    mask = np.asarray(mask)
    Wq, bq = np.asarray(Wq), np.asarray(bq)
    Wk, bk = np.asarray(Wk), np.asarray(bk)
    Wv, bv = np.asarray(Wv), np.asarray(bv)
    Wo, bo = np.asarray(Wo), np.asarray(bo)

    nc = get_program()
    in_maps = shard_inputs(x, mask, Wq, bq, Wk, bk, Wv, bv, Wo, bo)
    res = run_bass_kernel_spmd(nc, in_maps, core_ids=list(range(N_CORES)), **run_kwargs)
    # bias term that commutes with the cross-core reduction
    bias_term = (bo.astype(np.float32) + Wo.astype(np.float32) @ bv.astype(np.float32))
    out = gather_outputs(res.results, bias_term)
    kernel.last_results = res
    return out


# revision 41
# speedup vs baseline: 1.0022x; 1.0009x over previous
"""Causal multi-head attention on 8 Trainium2 NeuronCores.

Sharding: data-parallel over batch (B=2) x tensor-parallel over heads
(16 heads -> 4 groups of 4). Core c handles batch c//4, heads
[4*(c%4), 4*(c%4)+4). Each core computes its head-slice QKV projections,
causal softmax attention, and a partial output projection (row-sharded
Wo). The host sums the 4 partials per batch and adds the biases that
commute with the reduction (bo + Wo @ bv).

Schedule (v3, ~157us vs 173us baseline):
  - x is DMA'd q-chunk-major (one 8KB-run DMA per chunk, split over 4
    queues) and Q/K are projected one 512-column chunk at a time, so
    attention for chunk 0 starts ~8us in instead of ~30us.
  - the attention inner loop is ACT(exp)-bound; the remaining Q/K
    chunks, the next chunk's v projection and the previous chunk's
    output projection are threaded INTO the attention stream as
    128-mode "filler" matmuls after each p@v burst, keeping the PE
    busy so the HAM clock gate never re-throttles (idle-heavy
    schedules measured 13-24us stretches at 1.2GHz).
  - p@v runs in fp8e4 DoubleRow: one matmul contracts a k-tile PAIR
    (2 weights per PE cell), halving p@v streaming time. exp writes
    pt straight to fp8. The first k-tile pair of chunk 0 stays bf16:
    rows with tiny softmax support (q<256) cannot average away fp8
    noise (measured 3e-2 rel err at row 0 with all-fp8, 1/sqrt(n)
    decay).
  - Q/K/V projections for chunks 1-2 also run fp8 DoubleRow (d-tile
    pairs, K=256/pass). Chunks 0 and 3 stay bf16: chunk 0 for the
    same small-support precision reason, chunk 3 deliberately, as
    cheap PE filler in the ACT-bound tail (thin filler there lets
    the HAM clock gate re-throttle the PE to 1.2GHz).
  - the last chunk's normalize copies and output evacuations run on
    the Scalar engine (idle at the tail) instead of the congested
    DVE queue.
  - diagonal-pair exps are merged into one 3D-AP activation; the
    second tile's stale+triangle region is zeroed with a 256-wide
    [zeros|triangle] mask.
  - the last f-tile's normalize multiply is split per 128 columns so
    the tail output projection starts per-block.
  - partial output is shipped bf16 (halves the out DMA).

Hardware notes baked into this design (all measured on-device):
  - 64x128 row-tiled T0/T8 matmul pairs DO run concurrently
    (108ns/MM) but only in homogeneous instruction streams; tiling-
    mode switches cost ~105ns PE drain each, so 64-mode scores and
    128-mode everything-else are batched.
  - gpsimd partition_broadcast silently fails for output base
    partition 64 (only base-0 tiles work).
  - DVE tensor_tensor cannot read two PSUM operands (one PSUM read
    port) and cannot shift partitions (tensor_copy can).
"""

import os

os.environ.setdefault("MYCRO_LOCAL_CACHE", "1")

import ml_dtypes
import numpy as np

import concourse.bass as bass
import concourse.tile as tile
from concourse import bacc, mybir
from concourse.bass import ds, ts
from concourse.bass_utils import run_bass_kernel_spmd

AF = mybir.ActivationFunctionType

B = 2
S = 2048
D = 1024
N_HEADS = 16
DH = 64
N_CORES = 8

HG = 4            # heads per core
FH = HG * DH      # 256 features per core
P = 128
NFT = FH // P     # 2 f-tiles per core
NDT = D // P      # 8 d_model tiles
QC = 512          # q chunk (moving free dim)
NQC = S // QC     # 4
KT = 128          # k tile (partition dim of sT)
NKT = S // KT     # 16
NEH = D // QC     # 2 output-projection column halves

F32 = mybir.dt.float32
BF16 = mybir.dt.bfloat16
FP8 = mybir.dt.float8e4

MMDT = BF16
PVDT = FP8        # p@v operand dtype (DoubleRow: 2 k-tiles per matmul)
OUTDT = BF16      # partial-output dtype shipped to host
VPAD = 68         # v row padded so the DoubleRow weight step is 16B-aligned


def to_mmdt(a):
    a = np.ascontiguousarray(np.asarray(a, np.float32))
    return np.ascontiguousarray(a.astype(ml_dtypes.bfloat16))


def to_fp8(a):
    a = np.ascontiguousarray(np.asarray(a, np.float32))
    return np.ascontiguousarray(a.astype(ml_dtypes.float8_e4m3fn))


def build_program():
    nc = bacc.Bacc(None, target_bir_lowering=False)

    x_d = nc.dram_tensor("x_img", [P, NDT * S], MMDT, kind="ExternalInput")
    wq_d = nc.dram_tensor("wq_img", [P, NDT * FH], MMDT, kind="ExternalInput")
    wk_d = nc.dram_tensor("wk_img", [P, NDT * FH], MMDT, kind="ExternalInput")
    wv_d = nc.dram_tensor("wv_img", [P, NDT * FH], MMDT, kind="ExternalInput")
    wo_d = nc.dram_tensor("wo_img", [P, NFT * D], MMDT, kind="ExternalInput")
    x8_d = nc.dram_tensor("x8_img", [P, NDT * S], FP8, kind="ExternalInput")
    wq8_d = nc.dram_tensor("wq8_img", [P, NDT * FH], FP8, kind="ExternalInput")
    wk8_d = nc.dram_tensor("wk8_img", [P, NDT * FH], FP8, kind="ExternalInput")
    wv8_d = nc.dram_tensor("wv8_img", [P, NDT * FH], FP8, kind="ExternalInput")
    bq_d = nc.dram_tensor("bq2", [P, NFT], F32, kind="ExternalInput")
    bk_d = nc.dram_tensor("bk2", [P, NFT], F32, kind="ExternalInput")
    out_d = nc.dram_tensor("out", [S, D], OUTDT, kind="ExternalOutput")

    with tile.TileContext(nc) as tc:
        with tc.tile_pool(name="persist", bufs=1) as persist:
            qT = persist.tile([P, NFT, S], MMDT)
            kT = persist.tile([P, NFT, S], MMDT)
            v_sb = persist.tile([P, NKT, HG, VPAD], PVDT)
            v_bf = persist.tile([P, 2, HG, DH + 1], MMDT)
            aTn = persist.tile([P, NFT, S], MMDT)
            wo_sb = persist.tile([P, NFT, D], MMDT)
            bq_sb = persist.tile([P, NFT], F32)
            bk_sb = persist.tile([P, NFT], F32)

            nc.scalar.dma_start(bq_sb[:], bq_d[:])
            nc.scalar.dma_start(bk_sb[:], bk_d[:])
            nc.vector.memset(v_sb[:, :, :, DH : DH + 1], 1.0)
            nc.vector.memset(v_bf[:, :, :, DH : DH + 1], 1.0)

            # triangle mask (keep k<=q) for the causal diagonal, and a
            # [zeros | triangle] double-width variant for the merged
            # diagonal-pair exp (zeroes the stale region + the triangle)
            tri = persist.tile([P, KT], PVDT)
            tri_bf = persist.tile([P, KT], MMDT)
            nc.vector.memset(tri[:], 1.0)
            nc.vector.memset(tri_bf[:], 1.0)
            nc.gpsimd.affine_select(
                out=tri_bf[:],
                in_=tri_bf[:],
                compare_op=mybir.AluOpType.is_ge,
                fill=0.0,
                base=0,
                channel_multiplier=-1,
                pattern=[[1, KT]],
            )
            nc.gpsimd.affine_select(
                out=tri[:],
                in_=tri[:],
                compare_op=mybir.AluOpType.is_ge,
                fill=0.0,
                base=0,
                channel_multiplier=-1,
                pattern=[[1, KT]],
            )
            warm_w = persist.tile([P, KT], MMDT)
            warm_x = persist.tile([P, QC], MMDT)
            nc.vector.memset(warm_w[:], 0.25)
            nc.vector.memset(warm_x[:], 0.25)
            tri2 = persist.tile([P, 2 * KT], PVDT)
            nc.vector.memset(tri2[:], 1.0)
            nc.gpsimd.affine_select(
                out=tri2[:],
                in_=tri2[:],
                compare_op=mybir.AluOpType.is_ge,
                fill=0.0,
                base=-KT,
                channel_multiplier=-1,
                pattern=[[1, 2 * KT]],
            )
            with tc.tile_pool(name="proj", bufs=1) as proj_pool:
                # per-dt weight tiles and chunk-major x tiles: the first
                # projection matmul waits on 64KB of weights + 128KB of x
                wq_dt = [proj_pool.tile([P, FH], MMDT, name=f"wq{dt}") for dt in range(NDT)]
                wk_dt = [proj_pool.tile([P, FH], MMDT, name=f"wk{dt}") for dt in range(NDT)]
                wv_sb = proj_pool.tile([P, NDT, FH], MMDT)
                nc.scalar.dma_start(wq_dt[0][:], wq_d[:, ts(0, FH)])
                nc.scalar.dma_start(wk_dt[0][:], wk_d[:, ts(0, FH)])
                x0 = proj_pool.tile([P, NDT, QC], MMDT, name="x0")
                for part in range(4):
                    nc.sync.dma_start(
                        x0[:, 2 * part : 2 * part + 2, :],
                        x_d[:, ds(2 * part * QC, 2 * QC)].rearrange(
                            "p (dt q) -> p dt q", q=QC
                        ),
                    )
                xbf = {0: x0}
                x8_ct = [None]
                for c in (1, 2):
                    xc8 = proj_pool.tile([P, NDT, QC], FP8, name=f"x8_{c}")
                    for part in range(2):
                        nc.sync.dma_start(
                            xc8[:, 4 * part : 4 * part + 4, :],
                            x8_d[:, ds((c * NDT + 4 * part) * QC, 4 * QC)].rearrange(
                                "p (dt q) -> p dt q", q=QC
                            ),
                        )
                    x8_ct.append(xc8)
                x3 = proj_pool.tile([P, NDT, QC], MMDT, name="x3")
                for part in range(4):
                    nc.sync.dma_start(
                        x3[:, 2 * part : 2 * part + 2, :],
                        x_d[:, ds((3 * NDT + 2 * part) * QC, 2 * QC)].rearrange(
                            "p (dt q) -> p dt q", q=QC
                        ),
                    )
                xbf[3] = x3
                x_ct = {c: [xbf[c][:, dt, :] for dt in range(NDT)] for c in (0, 3)}
                for dt in range(1, NDT):
                    nc.scalar.dma_start(wq_dt[dt][:], wq_d[:, ts(dt, FH)])
                    nc.scalar.dma_start(wk_dt[dt][:], wk_d[:, ts(dt, FH)])
                nc.scalar.dma_start(wv_sb[:], wv_d[:].rearrange("p (dt f) -> p dt f", f=FH))
                wq8_sb = proj_pool.tile([P, NDT, FH], FP8)
                wk8_sb = proj_pool.tile([P, NDT, FH], FP8)
                wv8_sb = proj_pool.tile([P, NDT, FH], FP8)
                nc.scalar.dma_start(wq8_sb[:], wq8_d[:].rearrange("p (dt f) -> p dt f", f=FH))
                nc.scalar.dma_start(wk8_sb[:], wk8_d[:].rearrange("p (dt f) -> p dt f", f=FH))
                nc.scalar.dma_start(wv8_sb[:], wv8_d[:].rearrange("p (dt f) -> p dt f", f=FH))
                nc.scalar.dma_start(wo_sb[:], wo_d[:].rearrange("p (ft e) -> p ft e", e=D))

                # ACT exp-table load well before the first attention exp
                warm = persist.tile([P, 16], F32)
                nc.vector.memset(warm[:], 0.0)
                nc.scalar.activation(warm[:], warm[:], AF.Exp)

                with (
                    tc.tile_pool(name="attn_sb", bufs=6) as ap_pool,
                    tc.tile_pool(name="psum_a", bufs=1, space=bass.MemorySpace.PSUM) as pa,
                    tc.tile_pool(name="norm", bufs=3) as norm_pool,
                    tc.tile_pool(name="out_sb", bufs=3) as ot_pool,
                ):
                    # ---- 128-mode work units, emitted either as a burst or
                    # threaded into the attention stream as PE filler ----

                    def qk_chunk_mms(c):
                        """Closures projecting q/k for column chunk c.

                        Chunk 0 runs bf16 (rows 0-511 have too little
                        softmax support to average away fp8 noise); later
                        chunks contract d-tile PAIRS in fp8 DoubleRow."""
                        mms = []
                        specs = (
                            ((wq_dt, wq8_sb), bq_sb, qT),
                            ((wk_dt, wk8_sb), bk_sb, kT),
                        )
                        for (w_dt, w8), b_sb, dst in specs:
                            for ft in range(NFT):
                                box = {}

                                def alloc(box=box, c=c, ft=ft):
                                    box["ps"] = pa.tile(
                                        [P, QC], F32, tag="fill", bufs=2,
                                        name=f"pq{c}_{ft}",
                                    )

                                if c in (0, NQC - 1):
                                    for dt in range(NDT):
                                        def mm(box=box, w_dt=w_dt, dt=dt, ft=ft, c=c):
                                            if dt == 0:
                                                box["alloc"]()
                                            nc.tensor.matmul(
                                                box["ps"][:],
                                                w_dt[dt][:, ts(ft, P)],
                                                x_ct[c][dt][:],
                                                start=(dt == 0),
                                                stop=(dt == NDT - 1),
                                            )
                                        box["alloc"] = alloc
                                        mms.append(mm)
                                else:
                                    for dp in range(NDT // 2):
                                        def mm(box=box, w8=w8, dp=dp, ft=ft, c=c):
                                            if dp == 0:
                                                box["alloc"]()
                                            nc.tensor.matmul(
                                                box["ps"][:],
                                                w8[:, 2 * dp : 2 * dp + 2, ts(ft, P)],
                                                x8_ct[c][:, 2 * dp : 2 * dp + 2, :],
                                                start=(dp == 0),
                                                stop=(dp == NDT // 2 - 1),
                                                perf_mode=mybir.MatmulPerfMode.DoubleRow,
                                            )
                                        box["alloc"] = alloc
                                        mms.append(mm)

                                def evac(box=box, b_sb=b_sb, dst=dst, ft=ft, c=c):
                                    nc.scalar.activation(
                                        dst[:, ft, ts(c, QC)],
                                        box["ps"][:],
                                        AF.Identity,
                                        bias=b_sb[:, ft : ft + 1],
                                    )
                                mms.append(evac)
                        return mms

                    def v_proj_mms(qc):
                        """Closures projecting v for chunk qc's k-tiles."""
                        mms = []
                        for kt in range(qc * (QC // KT), (qc + 1) * (QC // KT)):
                            box = {}

                            def alloc(box=box, kt=kt):
                                box["ps"] = pa.tile(
                                    [P, FH], F32, tag="fill", bufs=2, name=f"pv{kt}"
                                )

                            if qc in (0, NQC - 1):
                                for dt in range(NDT):
                                    def mm(box=box, kt=kt, dt=dt, qc=qc):
                                        if dt == 0:
                                            box["alloc"]()
                                        nc.tensor.matmul(
                                            box["ps"][:],
                                            x_ct[qc][dt][:, ts(kt % 4, KT)],
                                            wv_sb[:, dt, :],
                                            start=(dt == 0),
                                            stop=(dt == NDT - 1),
                                        )
                                    box["alloc"] = alloc
                                    mms.append(mm)
                            else:
                                for dp in range(NDT // 2):
                                    def mm(box=box, kt=kt, dp=dp, qc=qc):
                                        if dp == 0:
                                            box["alloc"]()
                                        nc.tensor.matmul(
                                            box["ps"][:],
                                            x8_ct[qc][:, 2 * dp : 2 * dp + 2, ts(kt % 4, KT)],
                                            wv8_sb[:, 2 * dp : 2 * dp + 2, :],
                                            start=(dp == 0),
                                            stop=(dp == NDT // 2 - 1),
                                            perf_mode=mybir.MatmulPerfMode.DoubleRow,
                                        )
                                    box["alloc"] = alloc
                                    mms.append(mm)

                            def evac(box=box, kt=kt):
                                nc.vector.tensor_copy(
                                    v_sb[:, kt, :, 0:DH],
                                    box["ps"][:].rearrange("p (h d) -> p h d", h=HG),
                                )
                                if kt < 2:
                                    nc.vector.tensor_copy(
                                        v_bf[:, kt, :, 0:DH],
                                        box["ps"][:].rearrange("p (h d) -> p h d", h=HG),
                                    )
                            mms.append(evac)
                        return mms

                    def out_proj_mms(qc, evac_on_act=False):
                        """Closures for a finished q-range's output projection."""
                        mms = []
                        for qb in range(qc * (QC // P), (qc + 1) * (QC // P)):
                            box = {}

                            def alloc(box=box, qb=qb):
                                box["ps"] = [
                                    pa.tile(
                                        [P, QC], F32, tag="fill", bufs=2,
                                        name=f"po{qb}_{eh}",
                                    )
                                    for eh in range(NEH)
                                ]

                            for eh in range(NEH):
                                for ft in range(NFT):
                                    def mm(box=box, qb=qb, eh=eh, ft=ft):
                                        if eh == 0 and ft == 0:
                                            box["alloc"]()
                                        nc.tensor.matmul(
                                            box["ps"][eh][:],
                                            aTn[:, ft, ts(qb, P)],
                                            wo_sb[:, ft, ts(eh, QC)],
                                            start=(ft == 0),
                                            stop=(ft == NFT - 1),
                                        )
                                    box["alloc"] = alloc
                                    mms.append(mm)

                            def evac(box=box, qb=qb):
                                ot = ot_pool.tile([P, D], OUTDT, tag="ot", name=f"ot{qb}")
                                for eh in range(NEH):
                                    if evac_on_act and eh == 0:
                                        nc.scalar.activation(
                                            ot[:, ts(eh, QC)], box["ps"][eh][:], AF.Identity
                                        )
                                    else:
                                        nc.vector.tensor_copy(ot[:, ts(eh, QC)], box["ps"][eh][:])
                                    if evac_on_act:
                                        nc.sync.dma_start(
                                            out_d[ts(qb, P), ts(eh, QC)], ot[:, ts(eh, QC)]
                                        )
                                if not evac_on_act:
                                    nc.sync.dma_start(out_d[ts(qb, P), :], ot[:])
                            mms.append(evac)
                        return mms

                    def emit(queue, n):
                        for _ in range(min(n, len(queue))):
                            queue.pop(0)()

                    wps = pa.tile([P, QC], F32, tag="fill", bufs=2, name="warmps")
                    for _ in range(36):
                        nc.tensor.matmul(
                            wps[:], warm_w[:], warm_x[:], start=True, stop=True
                        )

                    # chunk 0 + its v projection run as a plain burst (there
                    # is no attention to interleave with yet)
                    boot = qk_chunk_mms(0) + v_proj_mms(0)
                    emit(boot, len(boot))

                    qcs = list(range(NQC))
                    for qi, qc in enumerate(qcs):
                        nkt = (qc + 1) * (QC // KT)
                        # 128-mode filler for this chunk's attention stream:
                        # next q/k chunk, next v tiles, previous out columns
                        filler = []
                        if qi + 1 < NQC:
                            filler += qk_chunk_mms(qi + 1)
                            filler += v_proj_mms(qi + 1)
                        if qi == 1:
                            filler += out_proj_mms(qcs[0])
                        elif qi == NQC - 1:
                            filler += out_proj_mms(qcs[qi - 2])
                            filler += out_proj_mms(qcs[qi - 1])

                        for hp in range(NFT):
                            heads = (2 * hp, 2 * hp + 1)
                            psa = {
                                h: pa.tile([DH + 1, QC], F32, tag="psa", bufs=2, name=f"psa{h}_{qc}")
                                for h in heads
                            }
                            pending = []

                            def flush_pair(filler=filler):
                                # pop both heads' tiles of one k-tile pair
                                # and contract both k-tiles in ONE fp8
                                # DoubleRow matmul (2 weights per PE cell),
                                # then thread in some 128-mode filler
                                for _ in range(2):
                                    h_, pt_, cc_ = pending.pop(0)
                                    kt0_, _, c0a_ = cc_[0]
                                    if qc == 0 and kt0_ == 0:
                                        for u_, (kt_, t_, c0_) in enumerate(cc_):
                                            nc.tensor.matmul(
                                                psa[h_][:, ds(c0_, QC - c0_)],
                                                v_bf[:, kt_, h_, :],
                                                pt_[:, ds(u_ * QC + c0_, QC - c0_)],
                                                start=(kt_ == 0),
                                                stop=False,
                                            )
                                    else:
                                        pt3 = pt_[:].rearrange("p (u q) -> p u q", u=2)
                                        nc.tensor.matmul(
                                            psa[h_][:, ds(c0a_, QC - c0a_)],
                                            v_sb[:, kt0_ : kt0_ + 2, h_, 0 : DH + 1],
                                            pt3[:, :, c0a_:],
                                            start=(kt0_ == 0 and qc != 0),
                                            stop=(kt0_ == nkt - 2),
                                            perf_mode=mybir.MatmulPerfMode.DoubleRow,
                                        )
                                emit(filler, 4)

                            for ktp in range(0, nkt, 2):
                                cc = []
                                for u in (0, 1):
                                    kt = ktp + u
                                    t = kt - qc * (QC // KT)
                                    c0 = KT * t if t > 0 else 0
                                    cc.append((kt, t, c0))
                                tiles = {
                                    h: (
                                        pa.tile(
                                            [P, 2 * QC], F32, tag="pss", bufs=2,
                                            name=f"pss{h}_{qc}_{ktp}",
                                        ),
                                        ap_pool.tile(
                                            [P, 2 * QC],
                                            MMDT if (qc == 0 and ktp == 0) else PVDT,
                                            tag="pt", bufs=6,
                                            name=f"pt{h}_{qc}_{ktp}",
                                        ),
                                    )
                                    for h in heads
                                }
                                for u, (kt, t, c0) in enumerate(cc):
                                    for h in heads:
                                        pb = DH * (h % 2)
                                        pss, pt = tiles[h]
                                        nc.tensor.matmul(
                                            pss[:, ds(u * QC + c0, QC - c0)],
                                            kT[pb : pb + DH, hp, ts(kt, KT)],
                                            qT[pb : pb + DH, hp, ds(qc * QC + c0, QC - c0)],
                                            start=True,
                                            stop=True,
                                            tile_position=(pb, 0),
                                        )
                                for h in heads:
                                    pss, pt = tiles[h]
                                    c0a, c0b = cc[0][2], cc[1][2]
                                    if c0a == 0 and c0b == 0:
                                        nc.scalar.activation(pt[:], pss[:], AF.Exp)
                                        for u, (kt, t, c0) in enumerate(cc):
                                            if t >= 0:
                                                reg = pt[:, ds(u * QC + c0, KT)]
                                                nc.vector.tensor_mul(reg, reg, tri[:])
                                    elif qc == 0:
                                        for u, (kt, t, c0) in enumerate(cc):
                                            nc.scalar.activation(
                                                pt[:, ds(u * QC + c0, QC - c0)],
                                                pss[:, ds(u * QC + c0, QC - c0)],
                                                AF.Exp,
                                            )
                                            nc.vector.tensor_mul(
                                                pt[:, ds(u * QC + c0, KT)],
                                                pt[:, ds(u * QC + c0, KT)],
                                                tri_bf[:] if ktp == 0 else tri[:],
                                            )
                                        if ktp > 0 and c0b > c0a:
                                            nc.vector.memset(
                                                pt[:, ds(QC + c0a, c0b - c0a)], 0.0
                                            )
                                    else:
                                        # one exp covers both tiles from col
                                        # c0a; tile u1's stale+triangle region
                                        # is zeroed with the 256-wide mask
                                        v3 = pss[:].rearrange("p (u q) -> p u q", u=2)
                                        o3 = pt[:].rearrange("p (u q) -> p u q", u=2)
                                        nc.scalar.activation(
                                            o3[:, :, c0a:], v3[:, :, c0a:], AF.Exp
                                        )
                                        nc.vector.tensor_mul(
                                            pt[:, ds(c0a, KT)],
                                            pt[:, ds(c0a, KT)],
                                            tri[:],
                                        )
                                        nc.vector.tensor_mul(
                                            pt[:, ds(QC + c0a, 2 * KT)],
                                            pt[:, ds(QC + c0a, 2 * KT)],
                                            tri2[:],
                                        )
                                    pending.append((h, pt, cc))
                                while len(pending) > 4:
                                    flush_pair()
                            while pending:
                                flush_pair()

                            # ---- normalize (broadcast only ever writes
                            # base-partition-0 tiles); the last f-tile runs
                            # the whole chain per column half so the tail
                            # out-projection starts earlier ----
                            nt = {}
                            for h in heads:
                                nt[h] = (
                                    norm_pool.tile([DH, QC], F32, tag="araw", bufs=4, name=f"araw{h}_{qc}"),
                                    norm_pool.tile([1, QC], F32, tag="se", bufs=4, name=f"se{h}_{qc}"),
                                    norm_pool.tile([DH, QC], F32, tag="sebc", bufs=4, name=f"sebc{h}_{qc}"),
                                    norm_pool.tile([DH, QC], F32, tag="rec", bufs=4, name=f"rec{h}_{qc}"),
                                )
                            last_hp = qc == NQC - 1 and hp == NFT - 1
                            halves = ((0, QC // 2), (QC // 2, QC // 2)) if last_hp else ((0, QC),)
                            for off, w in halves:
                                sl = ds(off, w)
                                for h in heads:
                                    if qc == NQC - 1:
                                        nc.scalar.activation(
                                            nt[h][1][:, sl], psa[h][DH : DH + 1, sl], AF.Identity
                                        )
                                        nc.scalar.activation(
                                            nt[h][0][:, sl], psa[h][0:DH, sl], AF.Identity
                                        )
                                    else:
                                        nc.vector.tensor_copy(nt[h][1][:, sl], psa[h][DH : DH + 1, sl])
                                        nc.vector.tensor_copy(nt[h][0][:, sl], psa[h][0:DH, sl])
                                for h in heads:
                                    nc.gpsimd.partition_broadcast(nt[h][2][:, sl], nt[h][1][:, sl])
                                for h in heads:
                                    nc.vector.reciprocal_approx_fast(nt[h][3][:, sl], nt[h][2][:, sl])
                                for h in heads:
                                    pb = DH * (h % 2)
                                    nc.vector.tensor_mul(
                                        aTn[pb : pb + DH, hp, ds(qc * QC + off, w)],
                                        nt[h][0][:, sl],
                                        nt[h][3][:, sl],
                                    )
                        # drain leftover filler as a burst before the next
                        # q-chunk's attention
                        emit(filler, len(filler))

                    tail = out_proj_mms(qcs[-1], evac_on_act=True)
                    emit(tail, len(tail))

    nc.finalize()
    return nc


_NC_CACHE = {}


def get_program():
    if "nc" not in _NC_CACHE:
        _NC_CACHE["nc"] = build_program()
    return _NC_CACHE["nc"]


def _x_img(xb):
    """x [S, D] -> chunk-major image [P, (chunk, dt, qcols)]."""
    a = to_mmdt(xb.T)                       # [D, S]
    a = a.reshape(NDT, P, NQC, QC).transpose(1, 2, 0, 3)
    return np.ascontiguousarray(a.reshape(P, NDT * S))


def _x8_img(xb):
    """x [S, D] -> chunk-major fp8 image [P, (chunk, dt, qcols)]."""
    a = to_fp8(xb.T)
    a = a.reshape(NDT, P, NQC, QC).transpose(1, 2, 0, 3)
    return np.ascontiguousarray(a.reshape(P, NDT * S))


def _img8(a):
    """[NDT*P, F] -> partition-major fp8 image [P, NDT*F]."""
    a = to_fp8(a)
    return np.ascontiguousarray(
        a.reshape(NDT, P, FH).transpose(1, 0, 2).reshape(P, NDT * FH)
    )


def _img(a, nt):
    """[nt*P, F] -> partition-major SBUF image [P, nt*F]."""
    ntp, f = a.shape
    assert ntp == nt * P
    return np.ascontiguousarray(
        a.reshape(nt, P, f).transpose(1, 0, 2).reshape(P, nt * f)
    )


def shard_inputs(x, mask, Wq, bq, Wk, bk, Wv, bv, Wo, bo):
    del mask  # causality is structural in the kernel
    in_maps = []
    for c in range(N_CORES):
        b = c // 4
        g = c % 4
        fsl = slice(FH * g, FH * (g + 1))
        in_maps.append(
            {
                "x_img": _x_img(x[b]),
                "x8_img": _x8_img(x[b]),
                "wq8_img": _img8(Wq[fsl, :].T / 8.0),
                "wk8_img": _img8(Wk[fsl, :].T),
                "wv8_img": _img8(Wv[fsl, :].T),
                "wq_img": _img(to_mmdt(Wq[fsl, :].T / 8.0), NDT),
                "wk_img": _img(to_mmdt(Wk[fsl, :].T), NDT),
                "wv_img": _img(to_mmdt(Wv[fsl, :].T), NDT),
                "wo_img": _img(to_mmdt(Wo[:, fsl].T), NFT),
                "bq2": np.ascontiguousarray(
                    (bq[fsl] / 8.0).reshape(NFT, P).T.astype(np.float32)
                ),
                "bk2": np.ascontiguousarray(
                    bk[fsl].reshape(NFT, P).T.astype(np.float32)
                ),
            }
        )
    return in_maps


def gather_outputs(results, bias_term):
    out = np.zeros((B, S, D), dtype=np.float32)
    for b in range(B):
        acc = results[4 * b]["out"].astype(np.float32)
        for g in range(1, 4):
            acc = acc + results[4 * b + g]["out"].astype(np.float32)
        out[b] = acc + bias_term
    return out


def kernel(x, mask, Wq, bq, Wk, bk, Wv, bv, Wo, bo, **run_kwargs):
    x = np.asarray(x)
    mask = np.asarray(mask)
    Wq, bq = np.asarray(Wq), np.asarray(bq)
    Wk, bk = np.asarray(Wk), np.asarray(bk)
    Wv, bv = np.asarray(Wv), np.asarray(bv)
    Wo, bo = np.asarray(Wo), np.asarray(bo)

    nc = get_program()
    in_maps = shard_inputs(x, mask, Wq, bq, Wk, bk, Wv, bv, Wo, bo)
    res = run_bass_kernel_spmd(nc, in_maps, core_ids=list(range(N_CORES)), **run_kwargs)
    # bias term that commutes with the cross-core reduction
    bias_term = (bo.astype(np.float32) + Wo.astype(np.float32) @ bv.astype(np.float32))
    out = gather_outputs(res.results, bias_term)
    kernel.last_results = res
    return out


# revision 42
# speedup vs baseline: 1.0044x; 1.0023x over previous
"""Causal multi-head attention on 8 Trainium2 NeuronCores.

Sharding: data-parallel over batch (B=2) x tensor-parallel over heads
(16 heads -> 4 groups of 4). Core c handles batch c//4, heads
[4*(c%4), 4*(c%4)+4). Each core computes its head-slice QKV projections,
causal softmax attention, and a partial output projection (row-sharded
Wo). The host sums the 4 partials per batch and adds the biases that
commute with the reduction (bo + Wo @ bv).

Schedule (v3, ~157us vs 173us baseline):
  - x is DMA'd q-chunk-major (one 8KB-run DMA per chunk, split over 4
    queues) and Q/K are projected one 512-column chunk at a time, so
    attention for chunk 0 starts ~8us in instead of ~30us.
  - the attention inner loop is ACT(exp)-bound; the remaining Q/K
    chunks, the next chunk's v projection and the previous chunk's
    output projection are threaded INTO the attention stream as
    128-mode "filler" matmuls after each p@v burst, keeping the PE
    busy so the HAM clock gate never re-throttles (idle-heavy
    schedules measured 13-24us stretches at 1.2GHz).
  - p@v runs in fp8e4 DoubleRow: one matmul contracts a k-tile PAIR
    (2 weights per PE cell), halving p@v streaming time. exp writes
    pt straight to fp8. The first k-tile pair of chunk 0 stays bf16:
    rows with tiny softmax support (q<256) cannot average away fp8
    noise (measured 3e-2 rel err at row 0 with all-fp8, 1/sqrt(n)
    decay).
  - Q/K/V projections for chunks 1-2 also run fp8 DoubleRow (d-tile
    pairs, K=256/pass). Chunks 0 and 3 stay bf16: chunk 0 for the
    same small-support precision reason, chunk 3 deliberately, as
    cheap PE filler in the ACT-bound tail (thin filler there lets
    the HAM clock gate re-throttle the PE to 1.2GHz).
  - the last chunk's normalize copies and output evacuations run on
    the Scalar engine (idle at the tail) instead of the congested
    DVE queue.
  - diagonal-pair exps are merged into one 3D-AP activation; the
    second tile's stale+triangle region is zeroed with a 256-wide
    [zeros|triangle] mask.
  - the last f-tile's normalize multiply is split per 128 columns so
    the tail output projection starts per-block.
  - partial output is shipped bf16 (halves the out DMA).

Hardware notes baked into this design (all measured on-device):
  - 64x128 row-tiled T0/T8 matmul pairs DO run concurrently
    (108ns/MM) but only in homogeneous instruction streams; tiling-
    mode switches cost ~105ns PE drain each, so 64-mode scores and
    128-mode everything-else are batched.
  - gpsimd partition_broadcast silently fails for output base
    partition 64 (only base-0 tiles work).
  - DVE tensor_tensor cannot read two PSUM operands (one PSUM read
    port) and cannot shift partitions (tensor_copy can).
"""

import os

os.environ.setdefault("MYCRO_LOCAL_CACHE", "1")

import ml_dtypes
import numpy as np

import concourse.bass as bass
import concourse.tile as tile
from concourse import bacc, mybir
from concourse.bass import ds, ts
from concourse.bass_utils import run_bass_kernel_spmd

AF = mybir.ActivationFunctionType

B = 2
S = 2048
D = 1024
N_HEADS = 16
DH = 64
N_CORES = 8

HG = 4            # heads per core
FH = HG * DH      # 256 features per core
P = 128
NFT = FH // P     # 2 f-tiles per core
NDT = D // P      # 8 d_model tiles
QC = 512          # q chunk (moving free dim)
NQC = S // QC     # 4
KT = 128          # k tile (partition dim of sT)
NKT = S // KT     # 16
NEH = D // QC     # 2 output-projection column halves

F32 = mybir.dt.float32
BF16 = mybir.dt.bfloat16
FP8 = mybir.dt.float8e4

MMDT = BF16
PVDT = FP8        # p@v operand dtype (DoubleRow: 2 k-tiles per matmul)
OUTDT = BF16      # partial-output dtype shipped to host
VPAD = 68         # v row padded so the DoubleRow weight step is 16B-aligned


def to_mmdt(a):
    a = np.ascontiguousarray(np.asarray(a, np.float32))
    return np.ascontiguousarray(a.astype(ml_dtypes.bfloat16))


def to_fp8(a):
    a = np.ascontiguousarray(np.asarray(a, np.float32))
    return np.ascontiguousarray(a.astype(ml_dtypes.float8_e4m3fn))


def build_program():
    nc = bacc.Bacc(None, target_bir_lowering=False)

    x_d = nc.dram_tensor("x_img", [P, NDT * S], MMDT, kind="ExternalInput")
    wq_d = nc.dram_tensor("wq_img", [P, NDT * FH], MMDT, kind="ExternalInput")
    wk_d = nc.dram_tensor("wk_img", [P, NDT * FH], MMDT, kind="ExternalInput")
    wv_d = nc.dram_tensor("wv_img", [P, NDT * FH], MMDT, kind="ExternalInput")
    wo_d = nc.dram_tensor("wo_img", [P, NFT * D], MMDT, kind="ExternalInput")
    x8_d = nc.dram_tensor("x8_img", [P, NDT * S], FP8, kind="ExternalInput")
    wq8_d = nc.dram_tensor("wq8_img", [P, NDT * FH], FP8, kind="ExternalInput")
    wk8_d = nc.dram_tensor("wk8_img", [P, NDT * FH], FP8, kind="ExternalInput")
    wv8_d = nc.dram_tensor("wv8_img", [P, NDT * FH], FP8, kind="ExternalInput")
    bq_d = nc.dram_tensor("bq2", [P, NFT], F32, kind="ExternalInput")
    bk_d = nc.dram_tensor("bk2", [P, NFT], F32, kind="ExternalInput")
    out_d = nc.dram_tensor("out", [S, D], OUTDT, kind="ExternalOutput")

    with tile.TileContext(nc) as tc:
        with tc.tile_pool(name="persist", bufs=1) as persist:
            qT = persist.tile([P, NFT, S], MMDT)
            kT = persist.tile([P, NFT, S], MMDT)
            v_sb = persist.tile([P, NKT, HG, VPAD], PVDT)
            v_bf = persist.tile([P, 2, HG, DH + 1], MMDT)
            aTn = persist.tile([P, NFT, S], MMDT)
            wo_sb = persist.tile([P, NFT, D], MMDT)
            bq_sb = persist.tile([P, NFT], F32)
            bk_sb = persist.tile([P, NFT], F32)

            nc.scalar.dma_start(bq_sb[:], bq_d[:])
            nc.scalar.dma_start(bk_sb[:], bk_d[:])
            nc.vector.memset(v_sb[:, :, :, DH : DH + 1], 1.0)
            nc.vector.memset(v_bf[:, :, :, DH : DH + 1], 1.0)

            # triangle mask (keep k<=q) for the causal diagonal, and a
            # [zeros | triangle] double-width variant for the merged
            # diagonal-pair exp (zeroes the stale region + the triangle)
            tri = persist.tile([P, KT], PVDT)
            tri_bf = persist.tile([P, KT], MMDT)
            nc.vector.memset(tri[:], 1.0)
            nc.vector.memset(tri_bf[:], 1.0)
            nc.gpsimd.affine_select(
                out=tri_bf[:],
                in_=tri_bf[:],
                compare_op=mybir.AluOpType.is_ge,
                fill=0.0,
                base=0,
                channel_multiplier=-1,
                pattern=[[1, KT]],
            )
            nc.gpsimd.affine_select(
                out=tri[:],
                in_=tri[:],
                compare_op=mybir.AluOpType.is_ge,
                fill=0.0,
                base=0,
                channel_multiplier=-1,
                pattern=[[1, KT]],
            )
            warm_w = persist.tile([P, KT], MMDT)
            warm_x = persist.tile([P, QC], MMDT)
            nc.vector.memset(warm_w[:], 0.25)
            nc.vector.memset(warm_x[:], 0.25)
            tri2 = persist.tile([P, 2 * KT], PVDT)
            nc.vector.memset(tri2[:], 1.0)
            nc.gpsimd.affine_select(
                out=tri2[:],
                in_=tri2[:],
                compare_op=mybir.AluOpType.is_ge,
                fill=0.0,
                base=-KT,
                channel_multiplier=-1,
                pattern=[[1, 2 * KT]],
            )
            with tc.tile_pool(name="proj", bufs=1) as proj_pool:
                # per-dt weight tiles and chunk-major x tiles: the first
                # projection matmul waits on 64KB of weights + 128KB of x
                wq_dt = [proj_pool.tile([P, FH], MMDT, name=f"wq{dt}") for dt in range(NDT)]
                wk_dt = [proj_pool.tile([P, FH], MMDT, name=f"wk{dt}") for dt in range(NDT)]
                wv_sb = proj_pool.tile([P, NDT, FH], MMDT)
                nc.scalar.dma_start(wq_dt[0][:], wq_d[:, ts(0, FH)])
                nc.scalar.dma_start(wk_dt[0][:], wk_d[:, ts(0, FH)])
                x0 = proj_pool.tile([P, NDT, QC], MMDT, name="x0")
                for part in range(4):
                    nc.sync.dma_start(
                        x0[:, 2 * part : 2 * part + 2, :],
                        x_d[:, ds(2 * part * QC, 2 * QC)].rearrange(
                            "p (dt q) -> p dt q", q=QC
                        ),
                    )
                xbf = {0: x0}
                x8_ct = [None]
                for c in (1, 2):
                    xc8 = proj_pool.tile([P, NDT, QC], FP8, name=f"x8_{c}")
                    for part in range(2):
                        nc.sync.dma_start(
                            xc8[:, 4 * part : 4 * part + 4, :],
                            x8_d[:, ds((c * NDT + 4 * part) * QC, 4 * QC)].rearrange(
                                "p (dt q) -> p dt q", q=QC
                            ),
                        )
                    x8_ct.append(xc8)
                x3 = proj_pool.tile([P, NDT, QC], MMDT, name="x3")
                for part in range(4):
                    nc.sync.dma_start(
                        x3[:, 2 * part : 2 * part + 2, :],
                        x_d[:, ds((3 * NDT + 2 * part) * QC, 2 * QC)].rearrange(
                            "p (dt q) -> p dt q", q=QC
                        ),
                    )
                xbf[3] = x3
                x_ct = {c: [xbf[c][:, dt, :] for dt in range(NDT)] for c in (0, 3)}
                for dt in range(1, NDT):
                    nc.scalar.dma_start(wq_dt[dt][:], wq_d[:, ts(dt, FH)])
                    nc.scalar.dma_start(wk_dt[dt][:], wk_d[:, ts(dt, FH)])
                nc.scalar.dma_start(wv_sb[:], wv_d[:].rearrange("p (dt f) -> p dt f", f=FH))
                wq8_sb = proj_pool.tile([P, NDT, FH], FP8)
                wk8_sb = proj_pool.tile([P, NDT, FH], FP8)
                wv8_sb = proj_pool.tile([P, NDT, FH], FP8)
                nc.scalar.dma_start(wq8_sb[:], wq8_d[:].rearrange("p (dt f) -> p dt f", f=FH))
                nc.scalar.dma_start(wk8_sb[:], wk8_d[:].rearrange("p (dt f) -> p dt f", f=FH))
                nc.scalar.dma_start(wv8_sb[:], wv8_d[:].rearrange("p (dt f) -> p dt f", f=FH))
                nc.scalar.dma_start(wo_sb[:], wo_d[:].rearrange("p (ft e) -> p ft e", e=D))

                # ACT exp-table load well before the first attention exp
                warm = persist.tile([P, 16], F32)
                nc.vector.memset(warm[:], 0.0)
                nc.scalar.activation(warm[:], warm[:], AF.Exp)

                with (
                    tc.tile_pool(name="attn_sb", bufs=6) as ap_pool,
                    tc.tile_pool(name="psum_a", bufs=1, space=bass.MemorySpace.PSUM) as pa,
                    tc.tile_pool(name="norm", bufs=3) as norm_pool,
                    tc.tile_pool(name="out_sb", bufs=3) as ot_pool,
                ):
                    # ---- 128-mode work units, emitted either as a burst or
                    # threaded into the attention stream as PE filler ----

                    def qk_chunk_mms(c):
                        """Closures projecting q/k for column chunk c.

                        Chunk 0 runs bf16 (rows 0-511 have too little
                        softmax support to average away fp8 noise); later
                        chunks contract d-tile PAIRS in fp8 DoubleRow."""
                        mms = []
                        specs = (
                            ((wq_dt, wq8_sb), bq_sb, qT),
                            ((wk_dt, wk8_sb), bk_sb, kT),
                        )
                        for (w_dt, w8), b_sb, dst in specs:
                            for ft in range(NFT):
                                box = {}

                                def alloc(box=box, c=c, ft=ft):
                                    box["ps"] = pa.tile(
                                        [P, QC], F32, tag="fill", bufs=2,
                                        name=f"pq{c}_{ft}",
                                    )

                                if c in (0, NQC - 1):
                                    for dt in range(NDT):
                                        def mm(box=box, w_dt=w_dt, dt=dt, ft=ft, c=c):
                                            if dt == 0:
                                                box["alloc"]()
                                            nc.tensor.matmul(
                                                box["ps"][:],
                                                w_dt[dt][:, ts(ft, P)],
                                                x_ct[c][dt][:],
                                                start=(dt == 0),
                                                stop=(dt == NDT - 1),
                                            )
                                        box["alloc"] = alloc
                                        mms.append(mm)
                                else:
                                    for dp in range(NDT // 2):
                                        def mm(box=box, w8=w8, dp=dp, ft=ft, c=c):
                                            if dp == 0:
                                                box["alloc"]()
                                            nc.tensor.matmul(
                                                box["ps"][:],
                                                w8[:, 2 * dp : 2 * dp + 2, ts(ft, P)],
                                                x8_ct[c][:, 2 * dp : 2 * dp + 2, :],
                                                start=(dp == 0),
                                                stop=(dp == NDT // 2 - 1),
                                                perf_mode=mybir.MatmulPerfMode.DoubleRow,
                                            )
                                        box["alloc"] = alloc
                                        mms.append(mm)

                                def evac(box=box, b_sb=b_sb, dst=dst, ft=ft, c=c):
                                    nc.scalar.activation(
                                        dst[:, ft, ts(c, QC)],
                                        box["ps"][:],
                                        AF.Identity,
                                        bias=b_sb[:, ft : ft + 1],
                                    )
                                mms.append(evac)
                        return mms

                    def v_proj_mms(qc):
                        """Closures projecting v for chunk qc's k-tiles."""
                        mms = []
                        for kt in range(qc * (QC // KT), (qc + 1) * (QC // KT)):
                            box = {}

                            def alloc(box=box, kt=kt):
                                box["ps"] = pa.tile(
                                    [P, FH], F32, tag="fill", bufs=2, name=f"pv{kt}"
                                )

                            if qc in (0, NQC - 1):
                                for dt in range(NDT):
                                    def mm(box=box, kt=kt, dt=dt, qc=qc):
                                        if dt == 0:
                                            box["alloc"]()
                                        nc.tensor.matmul(
                                            box["ps"][:],
                                            x_ct[qc][dt][:, ts(kt % 4, KT)],
                                            wv_sb[:, dt, :],
                                            start=(dt == 0),
                                            stop=(dt == NDT - 1),
                                        )
                                    box["alloc"] = alloc
                                    mms.append(mm)
                            else:
                                for dp in range(NDT // 2):
                                    def mm(box=box, kt=kt, dp=dp, qc=qc):
                                        if dp == 0:
                                            box["alloc"]()
                                        nc.tensor.matmul(
                                            box["ps"][:],
                                            x8_ct[qc][:, 2 * dp : 2 * dp + 2, ts(kt % 4, KT)],
                                            wv8_sb[:, 2 * dp : 2 * dp + 2, :],
                                            start=(dp == 0),
                                            stop=(dp == NDT // 2 - 1),
                                            perf_mode=mybir.MatmulPerfMode.DoubleRow,
                                        )
                                    box["alloc"] = alloc
                                    mms.append(mm)

                            def evac(box=box, kt=kt):
                                nc.vector.tensor_copy(
                                    v_sb[:, kt, :, 0:DH],
                                    box["ps"][:].rearrange("p (h d) -> p h d", h=HG),
                                )
                                if kt < 2:
                                    nc.vector.tensor_copy(
                                        v_bf[:, kt, :, 0:DH],
                                        box["ps"][:].rearrange("p (h d) -> p h d", h=HG),
                                    )
                            mms.append(evac)
                        return mms

                    def out_proj_mms(qc, evac_on_act=False):
                        """Closures for a finished q-range's output projection."""
                        mms = []
                        for qb in range(qc * (QC // P), (qc + 1) * (QC // P)):
                            box = {}

                            def alloc(box=box, qb=qb):
                                box["ps"] = [
                                    pa.tile(
                                        [P, QC], F32, tag="fill", bufs=2,
                                        name=f"po{qb}_{eh}",
                                    )
                                    for eh in range(NEH)
                                ]

                            for eh in range(NEH):
                                for ft in range(NFT):
                                    def mm(box=box, qb=qb, eh=eh, ft=ft):
                                        if eh == 0 and ft == 0:
                                            box["alloc"]()
                                        nc.tensor.matmul(
                                            box["ps"][eh][:],
                                            aTn[:, ft, ts(qb, P)],
                                            wo_sb[:, ft, ts(eh, QC)],
                                            start=(ft == 0),
                                            stop=(ft == NFT - 1),
                                        )
                                    box["alloc"] = alloc
                                    mms.append(mm)

                            def evac(box=box, qb=qb):
                                ot = ot_pool.tile([P, D], OUTDT, tag="ot", name=f"ot{qb}")
                                for eh in range(NEH):
                                    if evac_on_act and eh == 0:
                                        nc.scalar.activation(
                                            ot[:, ts(eh, QC)], box["ps"][eh][:], AF.Identity
                                        )
                                    else:
                                        nc.vector.tensor_copy(ot[:, ts(eh, QC)], box["ps"][eh][:])
                                    if evac_on_act:
                                        nc.sync.dma_start(
                                            out_d[ts(qb, P), ts(eh, QC)], ot[:, ts(eh, QC)]
                                        )
                                if not evac_on_act:
                                    nc.sync.dma_start(out_d[ts(qb, P), :], ot[:])
                            mms.append(evac)
                        return mms

                    def emit(queue, n):
                        for _ in range(min(n, len(queue))):
                            queue.pop(0)()

                    wps = pa.tile([P, QC], F32, tag="fill", bufs=2, name="warmps")
                    for _ in range(36):
                        nc.tensor.matmul(
                            wps[:], warm_w[:], warm_x[:], start=True, stop=True
                        )

                    # chunk 0 + its v projection run as a plain burst (there
                    # is no attention to interleave with yet)
                    boot = qk_chunk_mms(0) + v_proj_mms(0)
                    emit(boot, len(boot))

                    qcs = list(range(NQC))
                    for qi, qc in enumerate(qcs):
                        nkt = (qc + 1) * (QC // KT)
                        # 128-mode filler for this chunk's attention stream:
                        # next q/k chunk, next v tiles, previous out columns
                        filler = []
                        if qi + 1 < NQC:
                            filler += qk_chunk_mms(qi + 1)
                            filler += v_proj_mms(qi + 1)
                        if qi == 1:
                            filler += out_proj_mms(qcs[0])
                        elif qi == NQC - 1:
                            filler += out_proj_mms(qcs[qi - 2])
                            filler += out_proj_mms(qcs[qi - 1])

                        for hp in range(NFT):
                            heads = (2 * hp, 2 * hp + 1)
                            psa = {
                                h: pa.tile([DH + 1, QC], F32, tag="psa", bufs=2, name=f"psa{h}_{qc}")
                                for h in heads
                            }
                            pending = []

                            def flush_pair(filler=filler):
                                # pop both heads' tiles of one k-tile pair
                                # and contract both k-tiles in ONE fp8
                                # DoubleRow matmul (2 weights per PE cell),
                                # then thread in some 128-mode filler
                                for fi in range(2):
                                    emit(filler, 1 + fi)
                                    h_, pt_, cc_ = pending.pop(0)
                                    kt0_, _, c0a_ = cc_[0]
                                    if qc == 0 and kt0_ == 0:
                                        for u_, (kt_, t_, c0_) in enumerate(cc_):
                                            nc.tensor.matmul(
                                                psa[h_][:, ds(c0_, QC - c0_)],
                                                v_bf[:, kt_, h_, :],
                                                pt_[:, ds(u_ * QC + c0_, QC - c0_)],
                                                start=(kt_ == 0),
                                                stop=False,
                                            )
                                    else:
                                        pt3 = pt_[:].rearrange("p (u q) -> p u q", u=2)
                                        nc.tensor.matmul(
                                            psa[h_][:, ds(c0a_, QC - c0a_)],
                                            v_sb[:, kt0_ : kt0_ + 2, h_, 0 : DH + 1],
                                            pt3[:, :, c0a_:],
                                            start=(kt0_ == 0 and qc != 0),
                                            stop=(kt0_ == nkt - 2),
                                            perf_mode=mybir.MatmulPerfMode.DoubleRow,
                                        )
                                emit(filler, 1)

                            for ktp in range(0, nkt, 2):
                                cc = []
                                for u in (0, 1):
                                    kt = ktp + u
                                    t = kt - qc * (QC // KT)
                                    c0 = KT * t if t > 0 else 0
                                    cc.append((kt, t, c0))
                                tiles = {
                                    h: (
                                        pa.tile(
                                            [P, 2 * QC], F32, tag="pss", bufs=2,
                                            name=f"pss{h}_{qc}_{ktp}",
                                        ),
                                        ap_pool.tile(
                                            [P, 2 * QC],
                                            MMDT if (qc == 0 and ktp == 0) else PVDT,
                                            tag="pt", bufs=6,
                                            name=f"pt{h}_{qc}_{ktp}",
                                        ),
                                    )
                                    for h in heads
                                }
                                for u, (kt, t, c0) in enumerate(cc):
                                    for h in heads:
                                        pb = DH * (h % 2)
                                        pss, pt = tiles[h]
                                        nc.tensor.matmul(
                                            pss[:, ds(u * QC + c0, QC - c0)],
                                            kT[pb : pb + DH, hp, ts(kt, KT)],
                                            qT[pb : pb + DH, hp, ds(qc * QC + c0, QC - c0)],
                                            start=True,
                                            stop=True,
                                            tile_position=(pb, 0),
                                        )
                                for h in heads:
                                    pss, pt = tiles[h]
                                    c0a, c0b = cc[0][2], cc[1][2]
                                    if c0a == 0 and c0b == 0:
                                        nc.scalar.activation(pt[:], pss[:], AF.Exp)
                                        for u, (kt, t, c0) in enumerate(cc):
                                            if t >= 0:
                                                reg = pt[:, ds(u * QC + c0, KT)]
                                                nc.vector.tensor_mul(reg, reg, tri[:])
                                    elif qc == 0:
                                        for u, (kt, t, c0) in enumerate(cc):
                                            nc.scalar.activation(
                                                pt[:, ds(u * QC + c0, QC - c0)],
                                                pss[:, ds(u * QC + c0, QC - c0)],
                                                AF.Exp,
                                            )
                                            nc.vector.tensor_mul(
                                                pt[:, ds(u * QC + c0, KT)],
                                                pt[:, ds(u * QC + c0, KT)],
                                                tri_bf[:] if ktp == 0 else tri[:],
                                            )
                                        if ktp > 0 and c0b > c0a:
                                            nc.vector.memset(
                                                pt[:, ds(QC + c0a, c0b - c0a)], 0.0
                                            )
                                    else:
                                        # one exp covers both tiles from col
                                        # c0a; tile u1's stale+triangle region
                                        # is zeroed with the 256-wide mask
                                        v3 = pss[:].rearrange("p (u q) -> p u q", u=2)
                                        o3 = pt[:].rearrange("p (u q) -> p u q", u=2)
                                        nc.scalar.activation(
                                            o3[:, :, c0a:], v3[:, :, c0a:], AF.Exp
                                        )
                                        nc.vector.tensor_mul(
                                            pt[:, ds(c0a, KT)],
                                            pt[:, ds(c0a, KT)],
                                            tri[:],
                                        )
                                        nc.vector.tensor_mul(
                                            pt[:, ds(QC + c0a, 2 * KT)],
                                            pt[:, ds(QC + c0a, 2 * KT)],
                                            tri2[:],
                                        )
                                    pending.append((h, pt, cc))
                                while len(pending) > 4:
                                    flush_pair()
                            while pending:
                                flush_pair()

                            # ---- normalize (broadcast only ever writes
                            # base-partition-0 tiles); the last f-tile runs
                            # the whole chain per column half so the tail
                            # out-projection starts earlier ----
                            nt = {}
                            for h in heads:
                                nt[h] = (
                                    norm_pool.tile([DH, QC], F32, tag="araw", bufs=4, name=f"araw{h}_{qc}"),
                                    norm_pool.tile([1, QC], F32, tag="se", bufs=4, name=f"se{h}_{qc}"),
                                    norm_pool.tile([DH, QC], F32, tag="sebc", bufs=4, name=f"sebc{h}_{qc}"),
                                    norm_pool.tile([DH, QC], F32, tag="rec", bufs=4, name=f"rec{h}_{qc}"),
                                )
                            last_hp = qc == NQC - 1 and hp == NFT - 1
                            halves = ((0, QC // 2), (QC // 2, QC // 2)) if last_hp else ((0, QC),)
                            for off, w in halves:
                                sl = ds(off, w)
                                for h in heads:
                                    if qc == NQC - 1:
                                        nc.scalar.activation(
                                            nt[h][1][:, sl], psa[h][DH : DH + 1, sl], AF.Identity
                                        )
                                        nc.scalar.activation(
                                            nt[h][0][:, sl], psa[h][0:DH, sl], AF.Identity
                                        )
                                    else:
                                        nc.vector.tensor_copy(nt[h][1][:, sl], psa[h][DH : DH + 1, sl])
                                        nc.vector.tensor_copy(nt[h][0][:, sl], psa[h][0:DH, sl])
                                for h in heads:
                                    nc.gpsimd.partition_broadcast(nt[h][2][:, sl], nt[h][1][:, sl])
                                for h in heads:
                                    nc.vector.reciprocal_approx_fast(nt[h][3][:, sl], nt[h][2][:, sl])
                                for h in heads:
                                    pb = DH * (h % 2)
                                    nc.vector.tensor_mul(
                                        aTn[pb : pb + DH, hp, ds(qc * QC + off, w)],
                                        nt[h][0][:, sl],
                                        nt[h][3][:, sl],
                                    )
                        # drain leftover filler as a burst before the next
                        # q-chunk's attention
                        emit(filler, len(filler))

                    tail = out_proj_mms(qcs[-1], evac_on_act=True)
                    emit(tail, len(tail))

    nc.finalize()
    return nc


_NC_CACHE = {}


def get_program():
    if "nc" not in _NC_CACHE:
        _NC_CACHE["nc"] = build_program()
    return _NC_CACHE["nc"]


def _x_img(xb):
    """x [S, D] -> chunk-major image [P, (chunk, dt, qcols)]."""
    a = to_mmdt(xb.T)                       # [D, S]
    a = a.reshape(NDT, P, NQC, QC).transpose(1, 2, 0, 3)
    return np.ascontiguousarray(a.reshape(P, NDT * S))


def _x8_img(xb):
    """x [S, D] -> chunk-major fp8 image [P, (chunk, dt, qcols)]."""
    a = to_fp8(xb.T)
    a = a.reshape(NDT, P, NQC, QC).transpose(1, 2, 0, 3)
    return np.ascontiguousarray(a.reshape(P, NDT * S))


def _img8(a):
    """[NDT*P, F] -> partition-major fp8 image [P, NDT*F]."""
    a = to_fp8(a)
    return np.ascontiguousarray(
        a.reshape(NDT, P, FH).transpose(1, 0, 2).reshape(P, NDT * FH)
    )


def _img(a, nt):
    """[nt*P, F] -> partition-major SBUF image [P, nt*F]."""
    ntp, f = a.shape
    assert ntp == nt * P
    return np.ascontiguousarray(
        a.reshape(nt, P, f).transpose(1, 0, 2).reshape(P, nt * f)
    )


def shard_inputs(x, mask, Wq, bq, Wk, bk, Wv, bv, Wo, bo):
    del mask  # causality is structural in the kernel
    in_maps = []
    for c in range(N_CORES):
        b = c // 4
        g = c % 4
        fsl = slice(FH * g, FH * (g + 1))
        in_maps.append(
            {
                "x_img": _x_img(x[b]),
                "x8_img": _x8_img(x[b]),
                "wq8_img": _img8(Wq[fsl, :].T / 8.0),
                "wk8_img": _img8(Wk[fsl, :].T),
                "wv8_img": _img8(Wv[fsl, :].T),
                "wq_img": _img(to_mmdt(Wq[fsl, :].T / 8.0), NDT),
                "wk_img": _img(to_mmdt(Wk[fsl, :].T), NDT),
                "wv_img": _img(to_mmdt(Wv[fsl, :].T), NDT),
                "wo_img": _img(to_mmdt(Wo[:, fsl].T), NFT),
                "bq2": np.ascontiguousarray(
                    (bq[fsl] / 8.0).reshape(NFT, P).T.astype(np.float32)
                ),
                "bk2": np.ascontiguousarray(
                    bk[fsl].reshape(NFT, P).T.astype(np.float32)
                ),
            }
        )
    return in_maps


def gather_outputs(results, bias_term):
    out = np.zeros((B, S, D), dtype=np.float32)
    for b in range(B):
        acc = results[4 * b]["out"].astype(np.float32)
        for g in range(1, 4):
            acc = acc + results[4 * b + g]["out"].astype(np.float32)
        out[b] = acc + bias_term
    return out


def kernel(x, mask, Wq, bq, Wk, bk, Wv, bv, Wo, bo, **run_kwargs):
    x = np.asarray(x)
    mask = np.asarray(mask)
    Wq, bq = np.asarray(Wq), np.asarray(bq)
    Wk, bk = np.asarray(Wk), np.asarray(bk)
    Wv, bv = np.asarray(Wv), np.asarray(bv)
    Wo, bo = np.asarray(Wo), np.asarray(bo)

    nc = get_program()
    in_maps = shard_inputs(x, mask, Wq, bq, Wk, bk, Wv, bv, Wo, bo)
    res = run_bass_kernel_spmd(nc, in_maps, core_ids=list(range(N_CORES)), **run_kwargs)
    # bias term that commutes with the cross-core reduction
    bias_term = (bo.astype(np.float32) + Wo.astype(np.float32) @ bv.astype(np.float32))
    out = gather_outputs(res.results, bias_term)
    kernel.last_results = res
    return out


# revision 43
# speedup vs baseline: 1.0171x; 1.0126x over previous
"""Causal multi-head attention on 8 Trainium2 NeuronCores.

Sharding: data-parallel over batch (B=2) x tensor-parallel over heads
(16 heads -> 4 groups of 4). Core c handles batch c//4, heads
[4*(c%4), 4*(c%4)+4). Each core computes its head-slice QKV projections,
causal softmax attention, and a partial output projection (row-sharded
Wo). The host sums the 4 partials per batch and adds the biases that
commute with the reduction (bo + Wo @ bv).

Schedule (v3, ~157us vs 173us baseline):
  - x is DMA'd q-chunk-major (one 8KB-run DMA per chunk, split over 4
    queues) and Q/K are projected one 512-column chunk at a time, so
    attention for chunk 0 starts ~8us in instead of ~30us.
  - the attention inner loop is ACT(exp)-bound; the remaining Q/K
    chunks, the next chunk's v projection and the previous chunk's
    output projection are threaded INTO the attention stream as
    128-mode "filler" matmuls after each p@v burst, keeping the PE
    busy so the HAM clock gate never re-throttles (idle-heavy
    schedules measured 13-24us stretches at 1.2GHz).
  - p@v runs in fp8e4 DoubleRow: one matmul contracts a k-tile PAIR
    (2 weights per PE cell), halving p@v streaming time. exp writes
    pt straight to fp8. The first k-tile pair of chunk 0 stays bf16:
    rows with tiny softmax support (q<256) cannot average away fp8
    noise (measured 3e-2 rel err at row 0 with all-fp8, 1/sqrt(n)
    decay).
  - Q/K/V projections for chunks 1-2 also run fp8 DoubleRow (d-tile
    pairs, K=256/pass). Chunks 0 and 3 stay bf16: chunk 0 for the
    same small-support precision reason, chunk 3 deliberately, as
    cheap PE filler in the ACT-bound tail (thin filler there lets
    the HAM clock gate re-throttle the PE to 1.2GHz).
  - the last chunk's normalize copies and output evacuations run on
    the Scalar engine (idle at the tail) instead of the congested
    DVE queue.
  - diagonal-pair exps are merged into one 3D-AP activation; the
    second tile's stale+triangle region is zeroed with a 256-wide
    [zeros|triangle] mask.
  - the last f-tile's normalize multiply is split per 128 columns so
    the tail output projection starts per-block.
  - partial output is shipped bf16 (halves the out DMA).

Hardware notes baked into this design (all measured on-device):
  - 64x128 row-tiled T0/T8 matmul pairs DO run concurrently
    (108ns/MM) but only in homogeneous instruction streams; tiling-
    mode switches cost ~105ns PE drain each, so 64-mode scores and
    128-mode everything-else are batched.
  - gpsimd partition_broadcast silently fails for output base
    partition 64 (only base-0 tiles work).
  - DVE tensor_tensor cannot read two PSUM operands (one PSUM read
    port) and cannot shift partitions (tensor_copy can).
"""

import os

os.environ.setdefault("MYCRO_LOCAL_CACHE", "1")

import ml_dtypes
import numpy as np

import concourse.bass as bass
import concourse.tile as tile
from concourse import bacc, mybir
from concourse.bass import ds, ts
from concourse.bass_utils import run_bass_kernel_spmd

AF = mybir.ActivationFunctionType

B = 2
S = 2048
D = 1024
N_HEADS = 16
DH = 64
N_CORES = 8

HG = 4            # heads per core
FH = HG * DH      # 256 features per core
P = 128
NFT = FH // P     # 2 f-tiles per core
NDT = D // P      # 8 d_model tiles
QC = 512          # q chunk (moving free dim)
NQC = S // QC     # 4
KT = 128          # k tile (partition dim of sT)
NKT = S // KT     # 16
NEH = D // QC     # 2 output-projection column halves

F32 = mybir.dt.float32
BF16 = mybir.dt.bfloat16
FP8 = mybir.dt.float8e4

MMDT = BF16
PVDT = FP8        # p@v operand dtype (DoubleRow: 2 k-tiles per matmul)
OUTDT = BF16      # partial-output dtype shipped to host
VPAD = 68         # v row padded so the DoubleRow weight step is 16B-aligned


def to_mmdt(a):
    a = np.ascontiguousarray(np.asarray(a, np.float32))
    return np.ascontiguousarray(a.astype(ml_dtypes.bfloat16))


def to_fp8(a):
    a = np.ascontiguousarray(np.asarray(a, np.float32))
    return np.ascontiguousarray(a.astype(ml_dtypes.float8_e4m3fn))


def build_program():
    nc = bacc.Bacc(None, target_bir_lowering=False)

    x_d = nc.dram_tensor("x_img", [P, NDT * S], MMDT, kind="ExternalInput")
    wq_d = nc.dram_tensor("wq_img", [P, NDT * FH], MMDT, kind="ExternalInput")
    wk_d = nc.dram_tensor("wk_img", [P, NDT * FH], MMDT, kind="ExternalInput")
    wv_d = nc.dram_tensor("wv_img", [P, NDT * FH], MMDT, kind="ExternalInput")
    wo_d = nc.dram_tensor("wo_img", [P, NFT * D], MMDT, kind="ExternalInput")
    x8_d = nc.dram_tensor("x8_img", [P, NDT * S], FP8, kind="ExternalInput")
    wq8_d = nc.dram_tensor("wq8_img", [P, NDT * FH], FP8, kind="ExternalInput")
    wk8_d = nc.dram_tensor("wk8_img", [P, NDT * FH], FP8, kind="ExternalInput")
    wv8_d = nc.dram_tensor("wv8_img", [P, NDT * FH], FP8, kind="ExternalInput")
    bq_d = nc.dram_tensor("bq2", [P, NFT], F32, kind="ExternalInput")
    bk_d = nc.dram_tensor("bk2", [P, NFT], F32, kind="ExternalInput")
    out_d = nc.dram_tensor("out", [S, D], OUTDT, kind="ExternalOutput")

    with tile.TileContext(nc) as tc:
        with tc.tile_pool(name="persist", bufs=1) as persist:
            qT = persist.tile([P, NFT, S], MMDT)
            kT = persist.tile([P, NFT, S], MMDT)
            v_sb = persist.tile([P, NKT, HG, VPAD], PVDT)
            v_bf = persist.tile([P, 2, HG, DH + 1], MMDT)
            aTn = persist.tile([P, NFT, S], MMDT)
            wo_sb = persist.tile([P, NFT, D], MMDT)
            bq_sb = persist.tile([P, NFT], F32)
            bk_sb = persist.tile([P, NFT], F32)

            nc.scalar.dma_start(bq_sb[:], bq_d[:])
            nc.scalar.dma_start(bk_sb[:], bk_d[:])
            nc.vector.memset(v_sb[:, :, :, DH : DH + 1], 1.0)
            nc.vector.memset(v_bf[:, :, :, DH : DH + 1], 1.0)

            # triangle mask (keep k<=q) for the causal diagonal, and a
            # [zeros | triangle] double-width variant for the merged
            # diagonal-pair exp (zeroes the stale region + the triangle)
            tri = persist.tile([P, KT], PVDT)
            tri_bf = persist.tile([P, KT], MMDT)
            nc.vector.memset(tri[:], 1.0)
            nc.vector.memset(tri_bf[:], 1.0)
            nc.gpsimd.affine_select(
                out=tri_bf[:],
                in_=tri_bf[:],
                compare_op=mybir.AluOpType.is_ge,
                fill=0.0,
                base=0,
                channel_multiplier=-1,
                pattern=[[1, KT]],
            )
            nc.gpsimd.affine_select(
                out=tri[:],
                in_=tri[:],
                compare_op=mybir.AluOpType.is_ge,
                fill=0.0,
                base=0,
                channel_multiplier=-1,
                pattern=[[1, KT]],
            )
            warm_w = persist.tile([P, KT], MMDT)
            warm_x = persist.tile([P, QC], MMDT)
            nc.vector.memset(warm_w[:], 0.25)
            nc.vector.memset(warm_x[:], 0.25)
            tri2 = persist.tile([P, 2 * KT], PVDT)
            nc.vector.memset(tri2[:], 1.0)
            nc.gpsimd.affine_select(
                out=tri2[:],
                in_=tri2[:],
                compare_op=mybir.AluOpType.is_ge,
                fill=0.0,
                base=-KT,
                channel_multiplier=-1,
                pattern=[[1, 2 * KT]],
            )
            with tc.tile_pool(name="proj", bufs=1) as proj_pool:
                # per-dt weight tiles and chunk-major x tiles: the first
                # projection matmul waits on 64KB of weights + 128KB of x
                wq_dt = [proj_pool.tile([P, FH], MMDT, name=f"wq{dt}") for dt in range(NDT)]
                wk_dt = [proj_pool.tile([P, FH], MMDT, name=f"wk{dt}") for dt in range(NDT)]
                wv_sb = proj_pool.tile([P, NDT, FH], MMDT)
                nc.scalar.dma_start(wq_dt[0][:], wq_d[:, ts(0, FH)])
                nc.scalar.dma_start(wk_dt[0][:], wk_d[:, ts(0, FH)])
                x0 = proj_pool.tile([P, NDT, QC], MMDT, name="x0")
                for part in range(4):
                    nc.sync.dma_start(
                        x0[:, 2 * part : 2 * part + 2, :],
                        x_d[:, ds(2 * part * QC, 2 * QC)].rearrange(
                            "p (dt q) -> p dt q", q=QC
                        ),
                    )
                xbf = {0: x0}
                x8_ct = [None]
                for c in (1, 2):
                    xc8 = proj_pool.tile([P, NDT, QC], FP8, name=f"x8_{c}")
                    for part in range(2):
                        nc.sync.dma_start(
                            xc8[:, 4 * part : 4 * part + 4, :],
                            x8_d[:, ds((c * NDT + 4 * part) * QC, 4 * QC)].rearrange(
                                "p (dt q) -> p dt q", q=QC
                            ),
                        )
                    x8_ct.append(xc8)
                x3 = proj_pool.tile([P, NDT, QC], MMDT, name="x3")
                for part in range(4):
                    nc.sync.dma_start(
                        x3[:, 2 * part : 2 * part + 2, :],
                        x_d[:, ds((3 * NDT + 2 * part) * QC, 2 * QC)].rearrange(
                            "p (dt q) -> p dt q", q=QC
                        ),
                    )
                xbf[3] = x3
                x_ct = {c: [xbf[c][:, dt, :] for dt in range(NDT)] for c in (0, 3)}
                for dt in range(1, NDT):
                    nc.scalar.dma_start(wq_dt[dt][:], wq_d[:, ts(dt, FH)])
                    nc.scalar.dma_start(wk_dt[dt][:], wk_d[:, ts(dt, FH)])
                nc.scalar.dma_start(wv_sb[:], wv_d[:].rearrange("p (dt f) -> p dt f", f=FH))
                wq8_sb = proj_pool.tile([P, NDT, FH], FP8)
                wk8_sb = proj_pool.tile([P, NDT, FH], FP8)
                wv8_sb = proj_pool.tile([P, NDT, FH], FP8)
                nc.scalar.dma_start(wq8_sb[:], wq8_d[:].rearrange("p (dt f) -> p dt f", f=FH))
                nc.scalar.dma_start(wk8_sb[:], wk8_d[:].rearrange("p (dt f) -> p dt f", f=FH))
                nc.scalar.dma_start(wv8_sb[:], wv8_d[:].rearrange("p (dt f) -> p dt f", f=FH))
                nc.scalar.dma_start(wo_sb[:], wo_d[:].rearrange("p (ft e) -> p ft e", e=D))

                # ACT exp-table load well before the first attention exp
                warm = persist.tile([P, 16], F32)
                nc.vector.memset(warm[:], 0.0)
                nc.scalar.activation(warm[:], warm[:], AF.Exp)

                with (
                    tc.tile_pool(name="attn_sb", bufs=6) as ap_pool,
                    tc.tile_pool(name="psum_a", bufs=1, space=bass.MemorySpace.PSUM) as pa,
                    tc.tile_pool(name="norm", bufs=3) as norm_pool,
                    tc.tile_pool(name="out_sb", bufs=3) as ot_pool,
                ):
                    # ---- 128-mode work units, emitted either as a burst or
                    # threaded into the attention stream as PE filler ----

                    def qk_chunk_mms(c, fts=(0, 1)):
                        """Closures projecting q/k for column chunk c.

                        Chunk 0 runs bf16 (rows 0-511 have too little
                        softmax support to average away fp8 noise); later
                        chunks contract d-tile PAIRS in fp8 DoubleRow."""
                        mms = []
                        specs = (
                            ((wq_dt, wq8_sb), bq_sb, qT),
                            ((wk_dt, wk8_sb), bk_sb, kT),
                        )
                        for (w_dt, w8), b_sb, dst in specs:
                            for ft in fts:
                                box = {}

                                def alloc(box=box, c=c, ft=ft):
                                    box["ps"] = pa.tile(
                                        [P, QC], F32, tag="fill", bufs=2,
                                        name=f"pq{c}_{ft}",
                                    )

                                if c in (0, NQC - 1):
                                    for dt in range(NDT):
                                        def mm(box=box, w_dt=w_dt, dt=dt, ft=ft, c=c):
                                            if dt == 0:
                                                box["alloc"]()
                                            nc.tensor.matmul(
                                                box["ps"][:],
                                                w_dt[dt][:, ts(ft, P)],
                                                x_ct[c][dt][:],
                                                start=(dt == 0),
                                                stop=(dt == NDT - 1),
                                            )
                                        box["alloc"] = alloc
                                        mms.append(mm)
                                else:
                                    for dp in range(NDT // 2):
                                        def mm(box=box, w8=w8, dp=dp, ft=ft, c=c):
                                            if dp == 0:
                                                box["alloc"]()
                                            nc.tensor.matmul(
                                                box["ps"][:],
                                                w8[:, 2 * dp : 2 * dp + 2, ts(ft, P)],
                                                x8_ct[c][:, 2 * dp : 2 * dp + 2, :],
                                                start=(dp == 0),
                                                stop=(dp == NDT // 2 - 1),
                                                perf_mode=mybir.MatmulPerfMode.DoubleRow,
                                            )
                                        box["alloc"] = alloc
                                        mms.append(mm)

                                def evac(box=box, b_sb=b_sb, dst=dst, ft=ft, c=c):
                                    nc.scalar.activation(
                                        dst[:, ft, ts(c, QC)],
                                        box["ps"][:],
                                        AF.Identity,
                                        bias=b_sb[:, ft : ft + 1],
                                    )
                                mms.append(evac)
                        return mms

                    def v_proj_mms(qc):
                        """Closures projecting v for chunk qc's k-tiles."""
                        mms = []
                        for kt in range(qc * (QC // KT), (qc + 1) * (QC // KT)):
                            box = {}

                            def alloc(box=box, kt=kt):
                                box["ps"] = pa.tile(
                                    [P, FH], F32, tag="fill", bufs=2, name=f"pv{kt}"
                                )

                            if qc in (0, NQC - 1):
                                for dt in range(NDT):
                                    def mm(box=box, kt=kt, dt=dt, qc=qc):
                                        if dt == 0:
                                            box["alloc"]()
                                        nc.tensor.matmul(
                                            box["ps"][:],
                                            x_ct[qc][dt][:, ts(kt % 4, KT)],
                                            wv_sb[:, dt, :],
                                            start=(dt == 0),
                                            stop=(dt == NDT - 1),
                                        )
                                    box["alloc"] = alloc
                                    mms.append(mm)
                            else:
                                for dp in range(NDT // 2):
                                    def mm(box=box, kt=kt, dp=dp, qc=qc):
                                        if dp == 0:
                                            box["alloc"]()
                                        nc.tensor.matmul(
                                            box["ps"][:],
                                            x8_ct[qc][:, 2 * dp : 2 * dp + 2, ts(kt % 4, KT)],
                                            wv8_sb[:, 2 * dp : 2 * dp + 2, :],
                                            start=(dp == 0),
                                            stop=(dp == NDT // 2 - 1),
                                            perf_mode=mybir.MatmulPerfMode.DoubleRow,
                                        )
                                    box["alloc"] = alloc
                                    mms.append(mm)

                            def evac(box=box, kt=kt):
                                nc.vector.tensor_copy(
                                    v_sb[:, kt, :, 0:DH],
                                    box["ps"][:].rearrange("p (h d) -> p h d", h=HG),
                                )
                                if kt < 2:
                                    nc.vector.tensor_copy(
                                        v_bf[:, kt, :, 0:DH],
                                        box["ps"][:].rearrange("p (h d) -> p h d", h=HG),
                                    )
                            mms.append(evac)
                        return mms

                    def out_proj_mms(qc, evac_on_act=False):
                        """Closures for a finished q-range's output projection."""
                        mms = []
                        for qb in range(qc * (QC // P), (qc + 1) * (QC // P)):
                            box = {}

                            def alloc(box=box, qb=qb):
                                box["ps"] = [
                                    pa.tile(
                                        [P, QC], F32, tag="fill", bufs=2,
                                        name=f"po{qb}_{eh}",
                                    )
                                    for eh in range(NEH)
                                ]

                            for eh in range(NEH):
                                for ft in range(NFT):
                                    def mm(box=box, qb=qb, eh=eh, ft=ft):
                                        if eh == 0 and ft == 0:
                                            box["alloc"]()
                                        nc.tensor.matmul(
                                            box["ps"][eh][:],
                                            aTn[:, ft, ts(qb, P)],
                                            wo_sb[:, ft, ts(eh, QC)],
                                            start=(ft == 0),
                                            stop=(ft == NFT - 1),
                                        )
                                    box["alloc"] = alloc
                                    mms.append(mm)

                            def evac(box=box, qb=qb):
                                ot = ot_pool.tile([P, D], OUTDT, tag="ot", name=f"ot{qb}")
                                for eh in range(NEH):
                                    if evac_on_act and eh == 0:
                                        nc.scalar.activation(
                                            ot[:, ts(eh, QC)], box["ps"][eh][:], AF.Identity
                                        )
                                    else:
                                        nc.vector.tensor_copy(ot[:, ts(eh, QC)], box["ps"][eh][:])
                                    if evac_on_act:
                                        nc.sync.dma_start(
                                            out_d[ts(qb, P), ts(eh, QC)], ot[:, ts(eh, QC)]
                                        )
                                if not evac_on_act:
                                    nc.sync.dma_start(out_d[ts(qb, P), :], ot[:])
                            mms.append(evac)
                        return mms

                    def emit(queue, n):
                        for _ in range(min(n, len(queue))):
                            queue.pop(0)()

                    wps = pa.tile([P, QC], F32, tag="fill", bufs=2, name="warmps")
                    for _ in range(36):
                        nc.tensor.matmul(
                            wps[:], warm_w[:], warm_x[:], start=True, stop=True
                        )

                    # chunk 0: head-pair 0's attention only needs the ft0
                    # halves of qT/kT, so only those + v run as the boot
                    # burst; the ft1 groups thread into hp0's attention
                    boot = qk_chunk_mms(0, fts=(0,)) + v_proj_mms(0)
                    emit(boot, len(boot))
                    ft1_rest = qk_chunk_mms(0, fts=(1,))

                    qcs = list(range(NQC))
                    for qi, qc in enumerate(qcs):
                        nkt = (qc + 1) * (QC // KT)
                        # 128-mode filler for this chunk's attention stream:
                        # next q/k chunk, next v tiles, previous out columns
                        filler = []
                        if qi + 1 < NQC:
                            filler += qk_chunk_mms(qi + 1)
                            filler += v_proj_mms(qi + 1)
                        if qi == 1:
                            filler += out_proj_mms(qcs[0])
                        elif qi == NQC - 1:
                            filler += out_proj_mms(qcs[qi - 2])
                            filler += out_proj_mms(qcs[qi - 1])

                        for hp in range(NFT):
                            if qi == 0 and hp == 1:
                                emit(ft1_rest, len(ft1_rest))
                            heads = (2 * hp, 2 * hp + 1)
                            psa = {
                                h: pa.tile([DH + 1, QC], F32, tag="psa", bufs=2, name=f"psa{h}_{qc}")
                                for h in heads
                            }
                            pending = []

                            fill_src = ft1_rest if (qi == 0 and hp == 0 and ft1_rest) else filler

                            def flush_pair(filler=fill_src):
                                # pop both heads' tiles of one k-tile pair
                                # and contract both k-tiles in ONE fp8
                                # DoubleRow matmul (2 weights per PE cell),
                                # then thread in some 128-mode filler
                                for fi in range(2):
                                    emit(filler, 1 + fi)
                                    h_, pt_, cc_ = pending.pop(0)
                                    kt0_, _, c0a_ = cc_[0]
                                    if qc == 0 and kt0_ == 0:
                                        for u_, (kt_, t_, c0_) in enumerate(cc_):
                                            nc.tensor.matmul(
                                                psa[h_][:, ds(c0_, QC - c0_)],
                                                v_bf[:, kt_, h_, :],
                                                pt_[:, ds(u_ * QC + c0_, QC - c0_)],
                                                start=(kt_ == 0),
                                                stop=False,
                                            )
                                    else:
                                        pt3 = pt_[:].rearrange("p (u q) -> p u q", u=2)
                                        nc.tensor.matmul(
                                            psa[h_][:, ds(c0a_, QC - c0a_)],
                                            v_sb[:, kt0_ : kt0_ + 2, h_, 0 : DH + 1],
                                            pt3[:, :, c0a_:],
                                            start=(kt0_ == 0 and qc != 0),
                                            stop=(kt0_ == nkt - 2),
                                            perf_mode=mybir.MatmulPerfMode.DoubleRow,
                                        )
                                emit(filler, 1)

                            for ktp in range(0, nkt, 2):
                                cc = []
                                for u in (0, 1):
                                    kt = ktp + u
                                    t = kt - qc * (QC // KT)
                                    c0 = KT * t if t > 0 else 0
                                    cc.append((kt, t, c0))
                                tiles = {
                                    h: (
                                        pa.tile(
                                            [P, 2 * QC], F32, tag="pss", bufs=2,
                                            name=f"pss{h}_{qc}_{ktp}",
                                        ),
                                        ap_pool.tile(
                                            [P, 2 * QC],
                                            MMDT if (qc == 0 and ktp == 0) else PVDT,
                                            tag="pt", bufs=6,
                                            name=f"pt{h}_{qc}_{ktp}",
                                        ),
                                    )
                                    for h in heads
                                }
                                for u, (kt, t, c0) in enumerate(cc):
                                    for h in heads:
                                        pb = DH * (h % 2)
                                        pss, pt = tiles[h]
                                        nc.tensor.matmul(
                                            pss[:, ds(u * QC + c0, QC - c0)],
                                            kT[pb : pb + DH, hp, ts(kt, KT)],
                                            qT[pb : pb + DH, hp, ds(qc * QC + c0, QC - c0)],
                                            start=True,
                                            stop=True,
                                            tile_position=(pb, 0),
                                        )
                                for h in heads:
                                    pss, pt = tiles[h]
                                    c0a, c0b = cc[0][2], cc[1][2]
                                    if c0a == 0 and c0b == 0:
                                        nc.scalar.activation(pt[:], pss[:], AF.Exp)
                                        for u, (kt, t, c0) in enumerate(cc):
                                            if t >= 0:
                                                reg = pt[:, ds(u * QC + c0, KT)]
                                                nc.vector.tensor_mul(reg, reg, tri[:])
                                    elif qc == 0:
                                        for u, (kt, t, c0) in enumerate(cc):
                                            nc.scalar.activation(
                                                pt[:, ds(u * QC + c0, QC - c0)],
                                                pss[:, ds(u * QC + c0, QC - c0)],
                                                AF.Exp,
                                            )
                                            nc.vector.tensor_mul(
                                                pt[:, ds(u * QC + c0, KT)],
                                                pt[:, ds(u * QC + c0, KT)],
                                                tri_bf[:] if ktp == 0 else tri[:],
                                            )
                                        if ktp > 0 and c0b > c0a:
                                            nc.vector.memset(
                                                pt[:, ds(QC + c0a, c0b - c0a)], 0.0
                                            )
                                    else:
                                        # one exp covers both tiles from col
                                        # c0a; tile u1's stale+triangle region
                                        # is zeroed with the 256-wide mask
                                        v3 = pss[:].rearrange("p (u q) -> p u q", u=2)
                                        o3 = pt[:].rearrange("p (u q) -> p u q", u=2)
                                        nc.scalar.activation(
                                            o3[:, :, c0a:], v3[:, :, c0a:], AF.Exp
                                        )
                                        nc.vector.tensor_mul(
                                            pt[:, ds(c0a, KT)],
                                            pt[:, ds(c0a, KT)],
                                            tri[:],
                                        )
                                        nc.vector.tensor_mul(
                                            pt[:, ds(QC + c0a, 2 * KT)],
                                            pt[:, ds(QC + c0a, 2 * KT)],
                                            tri2[:],
                                        )
                                    pending.append((h, pt, cc))
                                while len(pending) > 4:
                                    flush_pair()
                            while pending:
                                flush_pair()

                            # ---- normalize (broadcast only ever writes
                            # base-partition-0 tiles); the last f-tile runs
                            # the whole chain per column half so the tail
                            # out-projection starts earlier ----
                            nt = {}
                            for h in heads:
                                nt[h] = (
                                    norm_pool.tile([DH, QC], F32, tag="araw", bufs=4, name=f"araw{h}_{qc}"),
                                    norm_pool.tile([1, QC], F32, tag="se", bufs=4, name=f"se{h}_{qc}"),
                                    norm_pool.tile([DH, QC], F32, tag="sebc", bufs=4, name=f"sebc{h}_{qc}"),
                                    norm_pool.tile([DH, QC], F32, tag="rec", bufs=4, name=f"rec{h}_{qc}"),
                                )
                            last_hp = qc == NQC - 1 and hp == NFT - 1
                            halves = ((0, QC // 2), (QC // 2, QC // 2)) if last_hp else ((0, QC),)
                            for off, w in halves:
                                sl = ds(off, w)
                                for h in heads:
                                    if qc == NQC - 1:
                                        nc.scalar.activation(
                                            nt[h][1][:, sl], psa[h][DH : DH + 1, sl], AF.Identity
                                        )
                                        nc.scalar.activation(
                                            nt[h][0][:, sl], psa[h][0:DH, sl], AF.Identity
                                        )
                                    else:
                                        nc.vector.tensor_copy(nt[h][1][:, sl], psa[h][DH : DH + 1, sl])
                                        nc.vector.tensor_copy(nt[h][0][:, sl], psa[h][0:DH, sl])
                                for h in heads:
                                    nc.gpsimd.partition_broadcast(nt[h][2][:, sl], nt[h][1][:, sl])
                                for h in heads:
                                    nc.vector.reciprocal_approx_fast(nt[h][3][:, sl], nt[h][2][:, sl])
                                for h in heads:
                                    pb = DH * (h % 2)
                                    nc.vector.tensor_mul(
                                        aTn[pb : pb + DH, hp, ds(qc * QC + off, w)],
                                        nt[h][0][:, sl],
                                        nt[h][3][:, sl],
                                    )
                        # drain leftover filler as a burst before the next
                        # q-chunk's attention
                        emit(filler, len(filler))

                    tail = out_proj_mms(qcs[-1], evac_on_act=True)
                    emit(tail, len(tail))

    nc.finalize()
    return nc


_NC_CACHE = {}


def get_program():
    if "nc" not in _NC_CACHE:
        _NC_CACHE["nc"] = build_program()
    return _NC_CACHE["nc"]


def _x_img(xb):
    """x [S, D] -> chunk-major image [P, (chunk, dt, qcols)]."""
    a = to_mmdt(xb.T)                       # [D, S]
    a = a.reshape(NDT, P, NQC, QC).transpose(1, 2, 0, 3)
    return np.ascontiguousarray(a.reshape(P, NDT * S))


def _x8_img(xb):
    """x [S, D] -> chunk-major fp8 image [P, (chunk, dt, qcols)]."""
    a = to_fp8(xb.T)
    a = a.reshape(NDT, P, NQC, QC).transpose(1, 2, 0, 3)
    return np.ascontiguousarray(a.reshape(P, NDT * S))


def _img8(a):
    """[NDT*P, F] -> partition-major fp8 image [P, NDT*F]."""
    a = to_fp8(a)
    return np.ascontiguousarray(
        a.reshape(NDT, P, FH).transpose(1, 0, 2).reshape(P, NDT * FH)
    )


def _img(a, nt):
    """[nt*P, F] -> partition-major SBUF image [P, nt*F]."""
    ntp, f = a.shape
    assert ntp == nt * P
    return np.ascontiguousarray(
        a.reshape(nt, P, f).transpose(1, 0, 2).reshape(P, nt * f)
    )


def shard_inputs(x, mask, Wq, bq, Wk, bk, Wv, bv, Wo, bo):
    del mask  # causality is structural in the kernel
    in_maps = []
    for c in range(N_CORES):
        b = c // 4
        g = c % 4
        fsl = slice(FH * g, FH * (g + 1))
        in_maps.append(
            {
                "x_img": _x_img(x[b]),
                "x8_img": _x8_img(x[b]),
                "wq8_img": _img8(Wq[fsl, :].T / 8.0),
                "wk8_img": _img8(Wk[fsl, :].T),
                "wv8_img": _img8(Wv[fsl, :].T),
                "wq_img": _img(to_mmdt(Wq[fsl, :].T / 8.0), NDT),
                "wk_img": _img(to_mmdt(Wk[fsl, :].T), NDT),
                "wv_img": _img(to_mmdt(Wv[fsl, :].T), NDT),
                "wo_img": _img(to_mmdt(Wo[:, fsl].T), NFT),
                "bq2": np.ascontiguousarray(
                    (bq[fsl] / 8.0).reshape(NFT, P).T.astype(np.float32)
                ),
                "bk2": np.ascontiguousarray(
                    bk[fsl].reshape(NFT, P).T.astype(np.float32)
                ),
            }
        )
    return in_maps


def gather_outputs(results, bias_term):
    out = np.zeros((B, S, D), dtype=np.float32)
    for b in range(B):
        acc = results[4 * b]["out"].astype(np.float32)
        for g in range(1, 4):
            acc = acc + results[4 * b + g]["out"].astype(np.float32)
        out[b] = acc + bias_term
    return out


def kernel(x, mask, Wq, bq, Wk, bk, Wv, bv, Wo, bo, **run_kwargs):
    x = np.asarray(x)
    mask = np.asarray(mask)
    Wq, bq = np.asarray(Wq), np.asarray(bq)
    Wk, bk = np.asarray(Wk), np.asarray(bk)
    Wv, bv = np.asarray(Wv), np.asarray(bv)
    Wo, bo = np.asarray(Wo), np.asarray(bo)

    nc = get_program()
    in_maps = shard_inputs(x, mask, Wq, bq, Wk, bk, Wv, bv, Wo, bo)
    res = run_bass_kernel_spmd(nc, in_maps, core_ids=list(range(N_CORES)), **run_kwargs)
    # bias term that commutes with the cross-core reduction
    bias_term = (bo.astype(np.float32) + Wo.astype(np.float32) @ bv.astype(np.float32))
    out = gather_outputs(res.results, bias_term)
    kernel.last_results = res
    return out
